# Initial kernel scaffold
#
"""Trainium2 Bass kernel for nn_BottleneckBlock (Chebyshev GNN bottleneck block).

Math restructure (per Chebyshev layer, K=3):
    out = x W0 + (Lx) W1 + (2LLx - x) W2
        = x (W0 - W2) + L(x W1 + 2 L (x W2))          # layers 1, 2 (project-then-propagate)
Layer 3 keeps the standard recursion (T1 = L y, T2 = 2 L T1 - y) so every
sparse propagation runs at 32 channels; batch (B=2) is fused into table rows
of 64 f32 = 256 B.  Biases before BatchNorm cancel and are dropped.

Sharding: nodes split 8 ways (6144/core).  Per propagation:
  AllGather (full 49152x64 table, rows in a per-core permuted order so shard
  writes are single contiguous DMAs) -> dma_gather of 512B paired rows
  (idx = row>>1 fits int16; 1024 idxs/call, 4 SWDGE queues, ~3us/call) ->
  DVE parity-select+scale by normalized edge weights -> TensorE reduction:
  edges sorted by 128-node dst block, one-hot [128 edge x 128 dst] stationaries
  (built once on GPSIMD, streamed from DRAM) matmul-accumulate each block in
  PSUM -> result rows land directly in SBUF.  No scatter (HW dma_scatter_add
  races on duplicate destinations and is RMW-slow).
"""

import os
import numpy as np

NC = 8
N = 49152
B = 2
C_MID = 32
C_OUT = 128
EPS = 1e-5
S = N // NC           # 6144 nodes per core
SI = S // 128         # 48 dst blocks / interleave groups
GCALL = int(os.environ.get("BK_GCALL", "1024"))
NQ = 4                # SWDGE queues

_CACHE = {}


def _wrap16(idx):
    a = np.asarray(idx, np.int16).reshape(-1, 16).T
    return np.ascontiguousarray(np.tile(a, (8, 1)))


def _nw_tile(v):
    return np.ascontiguousarray(np.asarray(v, np.float32).reshape(-1, 128).T)


def _perm_row(node):
    """Global node id -> permuted table row (per-core block-interleaved)."""
    c = node // S
    nl = node % S
    return c * S + (nl % 128) * SI + nl // 128


def _host_prep(x, edge_index, edge_weight):
    src = np.asarray(edge_index[0], np.int64)
    dst = np.asarray(edge_index[1], np.int64)
    ew = np.asarray(edge_weight, np.float32)

    deg = np.bincount(src, weights=ew.astype(np.float64), minlength=N).astype(np.float32)
    dinv = np.where(deg > 0, 1.0 / np.sqrt(np.maximum(deg, 1e-30)), 0.0).astype(np.float32)
    nw = (-dinv[src] * ew * dinv[dst]).astype(np.float32)

    # per-core edges grouped by dst block; per-block chunk counts unified
    per_core = []
    for c in range(NC):
        sel = np.nonzero((dst >= c * S) & (dst < (c + 1) * S))[0]
        d_loc = (dst[sel] - c * S).astype(np.int64)
        order = np.argsort(d_loc // 128, kind="stable")
        per_core.append((sel[order], d_loc[order]))

    kb = np.zeros(SI, np.int64)  # chunks per block (unified across cores)
    for c in range(NC):
        _, d_loc = per_core[c]
        cnt = np.bincount(d_loc // 128, minlength=SI)
        kb = np.maximum(kb, -(-cnt // 128))
    kb = np.maximum(kb, 1)
    k_end = np.cumsum(kb)
    k_off = k_end - kb
    NCH = int(k_end[-1])
    blocks = [(int(k_off[b]), int(k_end[b])) for b in range(SI)]
    NCHG = -(-NCH // 8)
    L2 = NCH * 128
    L2g = -(-L2 // GCALL) * GCALL
    NCALL = L2g // GCALL

    in_maps = []
    for c in range(NC):
        sel, d_loc = per_core[c]
        g16 = np.zeros(L2g, np.int16)
        nwe = np.zeros(L2g, np.float32)
        nwo = np.zeros(L2g, np.float32)
        dcol = np.full((128, NCHG * 8), -1.0, np.float32)
        cnt = np.bincount(d_loc // 128, minlength=SI)
        eo = np.concatenate([[0], np.cumsum(cnt)])
        for b in range(SI):
            e_ids = sel[eo[b]:eo[b + 1]]
            dl = d_loc[eo[b]:eo[b + 1]]
            o = int(k_off[b]) * 128
            k = e_ids.size
            rowp = _perm_row(src[e_ids])
            g16[o:o + k] = (rowp >> 1).astype(np.int16)
            par = (rowp & 1).astype(bool)
            w = nw[e_ids]
            nwe[o:o + k] = np.where(~par, w, 0.0)
            nwo[o:o + k] = np.where(par, w, 0.0)
            colv = np.full(int(kb[b]) * 128, -1.0, np.float32)
            colv[:k] = (dl % 128).astype(np.float32)
            dcol[:, int(k_off[b]):int(k_end[b])] = colv.reshape(-1, 128).T
        sl = slice(c * S, (c + 1) * S)
        xs = np.asarray(x[:, sl, :], np.float32)          # [2, S, 128]
        xr = np.concatenate([xs[0], xs[1]], axis=1)       # [S, 256] fused rows
        xrt = np.ascontiguousarray(
            xr.reshape(SI, 128, 256).transpose(1, 0, 2))  # [128, SI, 256] tile layout
        in_maps.append({
            "gidx": _wrap16(g16),
            "nwe": _nw_tile(nwe),
            "nwo": _nw_tile(nwo),
            "dstcol": np.ascontiguousarray(dcol),
            "xT": np.ascontiguousarray(xs.transpose(0, 2, 1)),   # [2, 128, S]
            "xrt": xrt,
        })

    iota = np.ascontiguousarray(
        np.broadcast_to(np.arange(128, dtype=np.float32), (128, 128)))
    for m in in_maps:
        m["iota"] = iota

    meta = {"L2g": L2g, "NCALL": NCALL, "NCH": NCH, "NCHG": NCHG, "blocks": blocks}
    return in_maps, meta


def _pack_weights(W1, W2, W3, g1, be1, g2, be2, g3, be3):
    W1 = np.asarray(W1, np.float32)
    W2 = np.asarray(W2, np.float32)
    W3 = np.asarray(W3, np.float32)
    w1cat = np.concatenate([W1[0] - W1[2], W1[1], W1[2]], axis=1)  # [128, 96]

    def fuse(w):  # [ci, co] -> [2ci, 2co] block-diag over batch
        ci, co = w.shape
        out = np.zeros((2 * ci, 2 * co), np.float32)
        out[:ci, :co] = w
        out[ci:, co:] = w
        return out

    w2bundle = np.concatenate([fuse(W2[0] - W2[2]), fuse(W2[1]), fuse(W2[2])], axis=1)
    return {
        "w1cat": np.ascontiguousarray(w1cat),
        "w2bundle": np.ascontiguousarray(w2bundle),          # [64, 192]
        "w3a": np.ascontiguousarray(fuse(W3[0] - W3[2])),    # [64, 256]
        "w3b": np.ascontiguousarray(fuse(W3[1])),
        "w3c": np.ascontiguousarray(fuse(2.0 * W3[2])),
        "g1": np.asarray(g1, np.float32)[None, :], "be1": np.asarray(be1, np.float32)[None, :],
        "g2": np.asarray(g2, np.float32)[None, :], "be2": np.asarray(be2, np.float32)[None, :],
        "g3": np.asarray(g3, np.float32)[None, :], "be3": np.asarray(be3, np.float32)[None, :],
    }


def _build_program(meta, debug=False):
    import contextlib
    import concourse.bacc as bacc
    import concourse.mybir as mybir
    import concourse.tile as tile
    from concourse.library_config import mlp
    from concourse.masks import make_identity

    f32 = mybir.dt.float32
    bf16 = mybir.dt.bfloat16
    i16 = mybir.dt.int16
    AT = mybir.AluOpType
    L2g, NCALL, NCH, NCHG, blocks = (
        meta["L2g"], meta["NCALL"], meta["NCH"], meta["NCHG"], meta["blocks"])

    nc = bacc.Bacc("TRN2", target_bir_lowering=False, debug=False, num_devices=NC,
                   num_swdge_queues=NQ,
                   dynamic_dma_scratch_size=int(os.environ.get("BK_SCRATCH", "16384")))

    # ---- I/O ----
    gidx = nc.dram_tensor("gidx", [128, L2g // 16], i16, kind="ExternalInput")
    nwe_d = nc.dram_tensor("nwe", [128, L2g // 128], f32, kind="ExternalInput")
    nwo_d = nc.dram_tensor("nwo", [128, L2g // 128], f32, kind="ExternalInput")
    dstcol_d = nc.dram_tensor("dstcol", [128, NCHG * 8], f32, kind="ExternalInput")
    iota_d = nc.dram_tensor("iota", [128, 128], f32, kind="ExternalInput")
    xT = nc.dram_tensor("xT", [B, 128, S], f32, kind="ExternalInput")
    xrt = nc.dram_tensor("xrt", [128, SI, 256], f32, kind="ExternalInput")
    w1cat = nc.dram_tensor("w1cat", [128, 96], f32, kind="ExternalInput")
    w2bundle = nc.dram_tensor("w2bundle", [64, 192], f32, kind="ExternalInput")
    w3a_d = nc.dram_tensor("w3a", [64, 256], f32, kind="ExternalInput")
    w3b_d = nc.dram_tensor("w3b", [64, 256], f32, kind="ExternalInput")
    w3c_d = nc.dram_tensor("w3c", [64, 256], f32, kind="ExternalInput")
    gbe_w = {"g1": 32, "be1": 32, "g2": 32, "be2": 32, "g3": 128, "be3": 128}
    gbe = {nm: nc.dram_tensor(nm, [1, w], f32, kind="ExternalInput") for nm, w in gbe_w.items()}
    out_d = nc.dram_tensor("out", [128, SI, 256], f32, kind="ExternalOutput")

    dbg = {}
    if debug:
        for nm in ["dbg_v1", "dbg_p11", "dbg_q1", "dbg_o1", "dbg_z2", "dbg_z3"]:
            dbg[nm] = nc.dram_tensor(nm, [128, SI, 64], f32, kind="ExternalOutput")

    # ---- internal DRAM ----
    full = [nc.dram_tensor(f"full{i}", [N, 64], f32, addr_space="Shared") for i in range(6)]
    shard = [nc.dram_tensor(f"shard{i}", [S, 64], f32) for i in range(6)]
    st_in = [nc.dram_tensor(f"stin{i}", [1, 512], f32) for i in range(3)]
    st_out = [nc.dram_tensor(f"stout{i}", [1, 512], f32, addr_space="Shared") for i in range(3)]
    a1d = nc.dram_tensor("a1d", [2, 128, SI, 32], f32)
    u1d = nc.dram_tensor("u1d", [2, 128, SI, 32], f32)
    a2d = nc.dram_tensor("a2d", [128, SI, 64], f32)
    u2d = nc.dram_tensor("u2d", [128, SI, 64], f32)
    o3d = nc.dram_tensor("o3d", [128, SI, 256], f32)
    stat_d = nc.dram_tensor("stat_d", [NCHG, 128, 8, 128], bf16)

    RG = [list(range(NC))]

    def shard_tile_ap(i):
        return shard[i][:].rearrange("(p i) e -> p i e", p=128)

    with tile.TileContext(nc) as tc, contextlib.ExitStack() as ctx:
        const = ctx.enter_context(tc.tile_pool(name="const", bufs=1))
        sb = ctx.enter_context(tc.tile_pool(name="sb", bufs=1))
        gp = ctx.enter_context(tc.tile_pool(name="gp", bufs=8))
        hp = ctx.enter_context(tc.tile_pool(name="hp", bufs=8))
        sp = ctx.enter_context(tc.tile_pool(name="sp", bufs=4))
        wp = ctx.enter_context(tc.tile_pool(name="wp", bufs=2))
        tl = ctx.enter_context(tc.tile_pool(name="tl", bufs=2))
        pp = ctx.enter_context(tc.tile_pool(name="pp", bufs=2, space="PSUM"))
        pp1 = ctx.enter_context(tc.tile_pool(name="pp1", bufs=1, space="PSUM"))

        nc.gpsimd.load_library(mlp)

        ident = const.tile([128, 128], f32)
        make_identity(nc, ident[:])
        ones_k = const.tile([128, 1], f32)
        nc.vector.memset(ones_k[:], 1.0)
        ones_m = const.tile([1, 128], f32)
        nc.vector.memset(ones_m[:], 1.0)

        gidx_sb = const.tile([128, L2g // 16], i16)
        nwe_sb = const.tile([128, L2g // 128], f32)
        nwo_sb = const.tile([128, L2g // 128], f32)
        dcol_sb = const.tile([128, NCHG * 8], f32)
        iota_sb = const.tile([128, 128], f32)
        nc.sync.dma_start(gidx_sb[:], gidx[:])
        nc.sync.dma_start(nwe_sb[:], nwe_d[:])
        nc.sync.dma_start(nwo_sb[:], nwo_d[:])
        nc.sync.dma_start(dcol_sb[:], dstcol_d[:])
        nc.sync.dma_start(iota_sb[:], iota_d[:])

        w1_sb = const.tile([128, 96], f32)
        w2_sb = const.tile([64, 192], f32)
        w3a = const.tile([64, 256], f32)
        w3b = const.tile([64, 256], f32)
        w3c = const.tile([64, 256], f32)
        nc.sync.dma_start(w1_sb[:], w1cat[:])
        nc.sync.dma_start(w2_sb[:], w2bundle[:])
        nc.sync.dma_start(w3a[:], w3a_d[:])
        nc.sync.dma_start(w3b[:], w3b_d[:])
        nc.sync.dma_start(w3c[:], w3c_d[:])
        gbe_sb = {}
        for nm, w in gbe_w.items():
            t = const.tile([1, w], f32)
            nc.sync.dma_start(t[:], gbe[nm][:])
            gbe_sb[nm] = t

        # ---- one-hot stationaries, built once on GPSIMD ----
        for g in range(NCHG):
            bt = sp.tile([128, 8, 128], bf16, tag="bt")
            for j in range(8):
                ch = g * 8 + j
                nc.vector.tensor_scalar(
                    out=bt[:, j, :], in0=iota_sb[:], scalar1=dcol_sb[:, ch:ch + 1],
                    scalar2=None, op0=AT.is_equal)
            nc.sync.dma_start(stat_d[g], bt[:])

        # ---- propagation ----
        def prop(t_i, prows):
            t2 = full[t_i][:].rearrange("(a b) e -> a (b e)", b=2)  # [N/2, 128]
            Hs = []
            for w in range(NCALL):
                G = gp.tile([128, GCALL // 128, 128], f32, tag="G")
                nc.gpsimd.dma_gather(G[:], t2, gidx_sb[:, w * (GCALL // 16):(w + 1) * (GCALL // 16)],
                                     GCALL, GCALL, 128, queue_num=w % NQ)
                H = hp.tile([128, GCALL // 128, 64], bf16, tag="H")
                GC = GCALL // 128
                ws = slice(w * GC, (w + 1) * GC)
                nc.vector.tensor_tensor(
                    out=G[:, :, 0:64], in0=G[:, :, 0:64],
                    in1=nwe_sb[:, ws, None].to_broadcast([128, GC, 64]), op=AT.mult)
                nc.vector.tensor_tensor(
                    out=G[:, :, 64:128], in0=G[:, :, 64:128],
                    in1=nwo_sb[:, ws, None].to_broadcast([128, GC, 64]), op=AT.mult)
                nc.vector.tensor_tensor(out=H[:], in0=G[:, :, 0:64], in1=G[:, :, 64:128],
                                        op=AT.add)
                Hs.append(H)
            sts = []
            for g in range(NCHG):
                st = sp.tile([128, 8, 128], bf16, tag="bt")
                nc.sync.dma_start(st[:], stat_d[g])
                sts.append(st)
            for b, (k0, k1) in enumerate(blocks):
                ps = pp.tile([128, 64], f32, tag="red")
                GC = GCALL // 128
                for k in range(k0, k1):
                    nc.tensor.matmul(ps[:], lhsT=sts[k // 8][:, k % 8, :],
                                     rhs=Hs[k // GC][:, k % GC, :],
                                     start=(k == k0), stop=(k == k1 - 1))
                nc.vector.tensor_copy(out=prows[:, b, :], in_=ps[:])

        # ---- BatchNorm helpers ----
        def bn_coeffs(sums, cmid, g_t, be_t, st_i):
            F = 2 * cmid
            ps = pp1.tile([1, 512], f32, tag="bnps")
            nc.tensor.matmul(ps[:, 0:2 * F], lhsT=ones_k[:], rhs=sums[:, 0:2 * F],
                             start=True, stop=True)
            stt = sb.tile([1, 512], f32, tag="bnstt")
            nc.vector.tensor_copy(out=stt[:, 0:2 * F], in_=ps[:, 0:2 * F])
            if 2 * F < 512:
                nc.vector.memset(stt[:, 2 * F:], 0.0)
            nc.sync.dma_start(st_in[st_i][:], stt[:])
            nc.gpsimd.collective_compute(
                "AllReduce", AT.add, replica_groups=RG,
                ins=[st_in[st_i][:].opt()], outs=[st_out[st_i][:].opt()])
            stf = sb.tile([1, 512], f32, tag="bnstf")
            nc.sync.dma_start(stf[:], st_out[st_i][:])
            cs = sb.tile([1, 8 * cmid], f32, tag="bncs")
            nc.vector.tensor_tensor(out=cs[:, 0:cmid], in0=stf[:, 0:cmid],
                                    in1=stf[:, cmid:F], op=AT.add)
            nc.vector.tensor_tensor(out=cs[:, cmid:2 * cmid], in0=stf[:, F:F + cmid],
                                    in1=stf[:, F + cmid:2 * F], op=AT.add)
            inv_n = 1.0 / float(B * N)
            mu = cs[:, 4 * cmid:5 * cmid]
            nc.vector.tensor_scalar_mul(mu, cs[:, 0:cmid], inv_n)
            msq = cs[:, 5 * cmid:6 * cmid]
            nc.vector.tensor_scalar_mul(msq, cs[:, cmid:2 * cmid], inv_n)
            var = cs[:, 6 * cmid:7 * cmid]
            nc.vector.tensor_tensor(out=var, in0=mu, in1=mu, op=AT.mult)
            nc.vector.tensor_tensor(out=var, in0=msq, in1=var, op=AT.subtract)
            nc.vector.tensor_scalar_add(var, var, EPS)
            std = cs[:, 7 * cmid:8 * cmid]
            nc.scalar.sqrt(std, var)
            rstd = cs[:, 6 * cmid:7 * cmid]
            nc.vector.reciprocal(rstd, std)
            s_ = cs[:, 2 * cmid:3 * cmid]
            nc.vector.tensor_tensor(out=s_, in0=g_t[:], in1=rstd, op=AT.mult)
            o_ = cs[:, 3 * cmid:4 * cmid]
            nc.vector.tensor_tensor(out=o_, in0=mu, in1=s_, op=AT.mult)
            nc.vector.tensor_tensor(out=o_, in0=be_t[:], in1=o_, op=AT.subtract)
            sf = sb.tile([1, 512], f32, tag="bnsf")
            nc.vector.tensor_copy(out=sf[:, 0:cmid], in_=s_)
            nc.vector.tensor_copy(out=sf[:, cmid:F], in_=s_)
            nc.vector.tensor_copy(out=sf[:, F:F + cmid], in_=o_)
            nc.vector.tensor_copy(out=sf[:, F + cmid:2 * F], in_=o_)
            psb = pp1.tile([128, 512], f32, tag="bnpsb")
            nc.tensor.matmul(psb[:, 0:2 * F], lhsT=ones_m[:], rhs=sf[:, 0:2 * F],
                             start=True, stop=True)
            rep = sb.tile([128, 512], f32, tag="bnrep")
            nc.vector.tensor_copy(out=rep[:, 0:2 * F], in_=psb[:, 0:2 * F])
            return rep

        def bn_relu_rows(orows, cmid, g_t, be_t, st_i, out_tag):
            F = 2 * cmid
            sums = sb.tile([128, 512], f32, tag="bnsums")
            nc.vector.tensor_reduce(out=sums[:, 0:F], in_=orows[:].rearrange("p i c -> p c i"),
                                    axis=mybir.AxisListType.X, op=AT.add)
            nc.vector.memset(sums[:, F:2 * F], 0.0)
            for gq in range(SI // 8):
                sq = tl.tile([128, 8, F], f32, tag="bnsqc")
                nc.vector.tensor_tensor(out=sq[:], in0=orows[:, gq * 8:(gq + 1) * 8, :],
                                        in1=orows[:, gq * 8:(gq + 1) * 8, :], op=AT.mult)
                red2 = tl.tile([128, F], f32, tag="bnred2")
                nc.vector.tensor_reduce(out=red2[:], in_=sq[:].rearrange("p i c -> p c i"),
                                        axis=mybir.AxisListType.X, op=AT.add)
                nc.vector.tensor_tensor(out=sums[:, F:2 * F], in0=sums[:, F:2 * F],
                                        in1=red2[:], op=AT.add)
            rep = bn_coeffs(sums, cmid, g_t, be_t, st_i)
            zr = sb.tile([128, SI, F], f32, tag=out_tag)
            nc.vector.tensor_tensor(out=zr[:], in0=orows[:],
                                    in1=rep[:, None, 0:F].to_broadcast([128, SI, F]), op=AT.mult)
            nc.vector.tensor_tensor(out=zr[:], in0=zr[:],
                                    in1=rep[:, None, F:2 * F].to_broadcast([128, SI, F]), op=AT.add)
            nc.vector.tensor_scalar_max(zr[:], zr[:], 0.0)
            return zr

        # ================= Layer 1 dense =================
        for g in range(SI // 8):
            hA = wp.tile([128, 8, 64], f32, tag="hA")
            hU = wp.tile([128, 8, 64], f32, tag="hU")
            hV = wp.tile([128, 8, 64], f32, tag="hV")
            for b in range(B):
                xtb = wp.tile([128, 1024], f32, tag="xtb")
                nc.sync.dma_start(xtb[:], xT[b, :, g * 1024:(g + 1) * 1024])
                hold = wp.tile([128, 8, 96], f32, tag="hold1")
                for j in range(8):
                    psd = pp.tile([128, 256], f32, tag="dps")
                    nc.tensor.matmul(psd[:, 0:96], lhsT=xtb[:, j * 128:(j + 1) * 128],
                                     rhs=w1_sb[:], start=True, stop=True)
                    nc.vector.tensor_copy(out=hold[:, j, :], in_=psd[:, 0:96])
                bs = slice(b * 32, (b + 1) * 32)
                nc.vector.tensor_copy(out=hA[:, :, bs], in_=hold[:, :, 0:32])
                nc.vector.tensor_copy(out=hU[:, :, bs], in_=hold[:, :, 32:64])
                nc.vector.tensor_copy(out=hV[:, :, bs], in_=hold[:, :, 64:96])
            gs = slice(g * 8, (g + 1) * 8)
            nc.sync.dma_start(a1d[0, :, gs, :], hA[:, :, 0:32])
            nc.sync.dma_start(a1d[1, :, gs, :], hA[:, :, 32:64])
            nc.sync.dma_start(u1d[0, :, gs, :], hU[:, :, 0:32])
            nc.sync.dma_start(u1d[1, :, gs, :], hU[:, :, 32:64])
            nc.sync.dma_start(shard_tile_ap(0)[:, gs, :], hV[:])
        if debug:
            nc.sync.dma_start(dbg["dbg_v1"][:], shard_tile_ap(0))
        nc.gpsimd.collective_compute("AllGather", AT.bypass, replica_groups=RG,
                                     ins=[shard[0][:].opt()], outs=[full[0][:].opt()])
        p11 = sb.tile([128, SI, 64], f32, tag="P1")
        prop(0, p11)
        if debug:
            nc.sync.dma_start(dbg["dbg_p11"][:], p11[:])
        u1r = sb.tile([128, 2, SI, 32], f32, tag="U")
        nc.sync.dma_start(u1r[:], u1d[:].rearrange("b p i c -> p b i c"))
        q1 = sb.tile([128, SI, 64], f32, tag="Q")
        for b in range(B):
            bs = slice(b * 32, (b + 1) * 32)
            nc.vector.scalar_tensor_tensor(
                out=q1[:, :, bs], in0=p11[:, :, bs], scalar=2.0,
                in1=u1r[:, b, :, :], op0=AT.mult, op1=AT.add)
        nc.sync.dma_start(shard_tile_ap(1), q1[:])
        if debug:
            nc.sync.dma_start(dbg["dbg_q1"][:], q1[:])
        nc.gpsimd.collective_compute("AllGather", AT.bypass, replica_groups=RG,
                                     ins=[shard[1][:].opt()], outs=[full[1][:].opt()])
        p12 = sb.tile([128, SI, 64], f32, tag="P1")
        prop(1, p12)
        a1r = sb.tile([128, 2, SI, 32], f32, tag="U")
        nc.sync.dma_start(a1r[:], a1d[:].rearrange("b p i c -> p b i c"))
        o1 = sb.tile([128, SI, 64], f32, tag="O")
        for b in range(B):
            bs = slice(b * 32, (b + 1) * 32)
            nc.vector.tensor_tensor(out=o1[:, :, bs], in0=p12[:, :, bs],
                                    in1=a1r[:, b, :, :], op=AT.add)
        if debug:
            nc.sync.dma_start(dbg["dbg_o1"][:], o1[:])
        z2 = bn_relu_rows(o1, C_MID, gbe_sb["g1"], gbe_sb["be1"], 0, "Z")
        if debug:
            nc.sync.dma_start(dbg["dbg_z2"][:], z2[:])

        # ================= Layer 2 =================
        for g in range(SI // 8):
            hold = wp.tile([128, 8, 192], f32, tag="hold2")
            for j in range(8):
                i = g * 8 + j
                tp = pp.tile([64, 128], f32, tag="tps")
                nc.tensor.transpose(out=tp[:], in_=z2[:, i, :], identity=ident[:])
                ztc = tl.tile([64, 128], f32, tag="ztc")
                nc.vector.tensor_copy(out=ztc[:], in_=tp[:])
                psd = pp.tile([128, 256], f32, tag="dps")
                nc.tensor.matmul(psd[:, 0:192], lhsT=ztc[:], rhs=w2_sb[:], start=True, stop=True)
                nc.vector.tensor_copy(out=hold[:, j, :], in_=psd[:, 0:192])
            gs = slice(g * 8, (g + 1) * 8)
            nc.sync.dma_start(a2d[:, gs, :], hold[:, :, 0:64])
            nc.sync.dma_start(u2d[:, gs, :], hold[:, :, 64:128])
            nc.sync.dma_start(shard_tile_ap(2)[:, gs, :], hold[:, :, 128:192])
        nc.gpsimd.collective_compute("AllGather", AT.bypass, replica_groups=RG,
                                     ins=[shard[2][:].opt()], outs=[full[2][:].opt()])
        p21 = sb.tile([128, SI, 64], f32, tag="P1")
        prop(2, p21)
        u2r = sb.tile([128, SI, 64], f32, tag="U")
        nc.sync.dma_start(u2r[:], u2d[:])
        q2 = sb.tile([128, SI, 64], f32, tag="Q")
        nc.vector.scalar_tensor_tensor(out=q2[:], in0=p21[:], scalar=2.0, in1=u2r[:],
                                       op0=AT.mult, op1=AT.add)
        nc.sync.dma_start(shard_tile_ap(3), q2[:])
        nc.gpsimd.collective_compute("AllGather", AT.bypass, replica_groups=RG,
                                     ins=[shard[3][:].opt()], outs=[full[3][:].opt()])
        p22 = sb.tile([128, SI, 64], f32, tag="P1")
        prop(3, p22)
        a2r = sb.tile([128, SI, 64], f32, tag="U")
        nc.sync.dma_start(a2r[:], a2d[:])
        o2 = sb.tile([128, SI, 64], f32, tag="O")
        nc.vector.tensor_tensor(out=o2[:], in0=p22[:], in1=a2r[:], op=AT.add)
        z3 = bn_relu_rows(o2, C_MID, gbe_sb["g2"], gbe_sb["be2"], 1, "Z")
        if debug:
            nc.sync.dma_start(dbg["dbg_z3"][:], z3[:])

        # ================= Layer 3 =================
        nc.sync.dma_start(shard_tile_ap(4), z3[:])
        nc.gpsimd.collective_compute("AllGather", AT.bypass, replica_groups=RG,
                                     ins=[shard[4][:].opt()], outs=[full[4][:].opt()])
        t1r = sb.tile([128, SI, 64], f32, tag="P1")
        prop(4, t1r)
        nc.sync.dma_start(shard_tile_ap(5), t1r[:])
        nc.gpsimd.collective_compute("AllGather", AT.bypass, replica_groups=RG,
                                     ins=[shard[5][:].opt()], outs=[full[5][:].opt()])
        p32 = sb.tile([128, SI, 64], f32, tag="Q")
        prop(5, p32)

        acc_s = sb.tile([128, 512], f32, tag="bnsums")
        nc.vector.memset(acc_s[:], 0.0)
        for g in range(SI // 8):
            hold = wp.tile([128, 8, 256], f32, tag="hold3")
            for j in range(8):
                i = g * 8 + j
                psd = pp.tile([128, 256], f32, tag="dps")
                for (rows_t, w_t, st_, sp_) in ((z3, w3a, True, False),
                                                (t1r, w3b, False, False),
                                                (p32, w3c, False, True)):
                    tp = pp.tile([64, 128], f32, tag="tps")
                    nc.tensor.transpose(out=tp[:], in_=rows_t[:, i, :], identity=ident[:])
                    ztc = tl.tile([64, 128], f32, tag="ztc")
                    nc.vector.tensor_copy(out=ztc[:], in_=tp[:])
                    nc.tensor.matmul(psd[:], lhsT=ztc[:], rhs=w_t[:], start=st_, stop=sp_)
                nc.vector.tensor_copy(out=hold[:, j, :], in_=psd[:])
            nc.sync.dma_start(o3d[:, g * 8:(g + 1) * 8, :], hold[:])
            red = sb.tile([128, 512], f32, tag="red")
            nc.vector.tensor_reduce(out=red[:, 0:256], in_=hold[:].rearrange("p j c -> p c j"),
                                    axis=mybir.AxisListType.X, op=AT.add)
            sqh = wp.tile([128, 8, 256], f32, tag="hold3")
            nc.vector.tensor_tensor(out=sqh[:], in0=hold[:], in1=hold[:], op=AT.mult)
            nc.vector.tensor_reduce(out=red[:, 256:512], in_=sqh[:].rearrange("p j c -> p c j"),
                                    axis=mybir.AxisListType.X, op=AT.add)
            nc.vector.tensor_tensor(out=acc_s[:], in0=acc_s[:], in1=red[:], op=AT.add)
        rep3 = bn_coeffs(acc_s, C_OUT, gbe_sb["g3"], gbe_sb["be3"], 2)

        for g in range(SI):
            gs = slice(g, g + 1)
            o3c = tl.tile([128, 1, 256], f32, tag="o3c")
            nc.sync.dma_start(o3c[:], o3d[:, gs, :])
            zc = tl.tile([128, 1, 256], f32, tag="zc")
            nc.vector.tensor_tensor(out=zc[:], in0=o3c[:],
                                    in1=rep3[:, None, 0:256].to_broadcast([128, 1, 256]),
                                    op=AT.mult)
            nc.vector.tensor_tensor(out=zc[:], in0=zc[:],
                                    in1=rep3[:, None, 256:512].to_broadcast([128, 1, 256]),
                                    op=AT.add)
            nc.vector.tensor_scalar_max(zc[:], zc[:], 0.0)
            xc = tl.tile([128, 1, 256], f32, tag="xc")
            nc.sync.dma_start(xc[:], xrt[:, gs, :])
            nc.vector.tensor_tensor(out=zc[:], in0=zc[:], in1=xc[:], op=AT.add)
            nc.vector.tensor_scalar_max(zc[:], zc[:], 0.0)
            nc.sync.dma_start(out_d[:, gs, :], zc[:])

    nc.compile()
    return nc


def kernel(x, edge_index, edge_weight,
           W1, b1, g1, be1, W2, b2, g2, be2, W3, b3, g3, be3):
    from concourse.bass_utils import run_bass_kernel_spmd

    x = np.asarray(x, np.float32)
    in_maps, meta = _host_prep(x, edge_index, edge_weight)
    wts = _pack_weights(W1, W2, W3, g1, be1, g2, be2, g3, be3)
    for m in in_maps:
        m.update(wts)

    debug = os.environ.get("BK_DEBUG", "0") == "1"
    key = (meta["L2g"], meta["NCH"], tuple(k for _, k in meta["blocks"]), debug)
    if key not in _CACHE:
        _CACHE[key] = _build_program(meta, debug=debug)
    nc = _CACHE[key]

    trace = os.environ.get("BK_TRACE", "0") == "1"
    kw = {"trace": True} if trace else {}
    res = run_bass_kernel_spmd(nc, in_maps, list(range(NC)), **kw)
    if trace:
        print(f"HW exec time: {res.exec_time_ns} ns (mean {res.mean_exec_time_ns})")

    out = np.empty((B, N, 128), np.float32)
    for c in range(NC):
        oc = res.results[c]["out"]  # [128, SI, 256] tile layout
        rows = oc.transpose(1, 0, 2).reshape(S, 256)  # node = i*128 + p
        out[0, c * S:(c + 1) * S, :] = rows[:, 0:128]
        out[1, c * S:(c + 1) * S, :] = rows[:, 128:256]
    kernel._last_results = res
    return out



# revision 6
# speedup vs baseline: 1.0572x; 1.0572x over previous
"""Trainium2 Bass kernel for nn_BottleneckBlock (Chebyshev GNN bottleneck block).

v2: memory-traffic-focused rewrite of the baseline.
  - All propagation tables, gathers, and intermediate tiles in bf16
    (halves gather + AllGather + table bytes).
  - One-hot dst stationaries built once on DVE and kept RESIDENT in SBUF
    (fp8 or bf16; baseline streamed 14MB/prop from DRAM).
  - Edge-weight multiply + parity select: 3 narrow bf16 DVE ops per
    gather call (baseline: 3 full-width f32 ops).
  - Dense layers bf16 (except layer 1, f32 inputs), single fused
    eviction copies on the Scalar engine, psum evictions on Scalar.
  - o3 round-trip and xrt residual in bf16.

Math restructure (per Chebyshev layer, K=3) as baseline:
    out = x (W0 - W2) + L(x W1 + 2 L (x W2))  for layers 1, 2
    layer 3 standard recursion.  Biases before BatchNorm cancel.
Sharding: nodes split 8 ways; per-prop AllGather of the bf16 table in
permuted row order (shard writes contiguous); dma_gather of paired
256B rows; TensorE one-hot reduction per 128-dst block.
"""

import os
import contextlib
import numpy as np

NC = 8
N = 49152
B = 2
C_MID = 32
C_OUT = 128
EPS = 1e-5
S = N // NC           # 6144 nodes per core
SI = S // 128         # 48 dst blocks
GCALL = int(os.environ.get("BK_GCALL", "2048"))
GC = GCALL // 128     # chunks per gather call
NQ = int(os.environ.get("BK_NQ", "4"))
STAT_FP8 = os.environ.get("BK_STATFP8", "1") == "1"

_CACHE = {}


def _wrap16(idx):
    a = np.asarray(idx, np.int16).reshape(-1, 16).T
    return np.ascontiguousarray(np.tile(a, (8, 1)))


def _perm_row(node):
    """Global node id -> permuted table row (per-core block-interleaved)."""
    c = node // S
    nl = node % S
    return c * S + (nl % 128) * SI + nl // 128


def _host_prep(x, edge_index, edge_weight):
    import ml_dtypes
    bf = ml_dtypes.bfloat16

    src = np.asarray(edge_index[0], np.int64)
    dst = np.asarray(edge_index[1], np.int64)
    ew = np.asarray(edge_weight, np.float32)

    deg = np.bincount(src, weights=ew.astype(np.float64), minlength=N).astype(np.float32)
    dinv = np.where(deg > 0, 1.0 / np.sqrt(np.maximum(deg, 1e-30)), 0.0).astype(np.float32)
    nw = (-dinv[src] * ew * dinv[dst]).astype(np.float32)

    per_core = []
    for c in range(NC):
        sel = np.nonzero((dst >= c * S) & (dst < (c + 1) * S))[0]
        d_loc = (dst[sel] - c * S).astype(np.int64)
        order = np.argsort(d_loc // 128, kind="stable")
        per_core.append((sel[order], d_loc[order]))

    kb = np.zeros(SI, np.int64)  # chunks per block (unified across cores)
    for c in range(NC):
        _, d_loc = per_core[c]
        cnt = np.bincount(d_loc // 128, minlength=SI)
        kb = np.maximum(kb, -(-cnt // 128))
    kb = np.maximum(kb, 1)
    k_end = np.cumsum(kb)
    k_off = k_end - kb
    NCH = int(k_end[-1])
    blocks = [(int(k_off[b]), int(k_end[b])) for b in range(SI)]
    L2 = NCH * 128
    L2g = -(-L2 // GCALL) * GCALL
    NCALL = L2g // GCALL
    NCHP = L2g // 128    # padded chunk count (we/wo sized to this)

    in_maps = []
    for c in range(NC):
        sel, d_loc = per_core[c]
        g16 = np.zeros(L2g, np.int16)
        nwe = np.zeros(L2g, np.float32)
        nwo = np.zeros(L2g, np.float32)
        dcol = np.full((128, NCHP), -1.0, np.float32)
        cnt = np.bincount(d_loc // 128, minlength=SI)
        eo = np.concatenate([[0], np.cumsum(cnt)])
        for b in range(SI):
            e_ids = sel[eo[b]:eo[b + 1]]
            dl = d_loc[eo[b]:eo[b + 1]]
            o = int(k_off[b]) * 128
            k = e_ids.size
            rowp = _perm_row(src[e_ids])
            g16[o:o + k] = (rowp >> 1).astype(np.int16)
            par = (rowp & 1).astype(bool)
            w = nw[e_ids]
            nwe[o:o + k] = np.where(~par, w, 0.0)
            nwo[o:o + k] = np.where(par, w, 0.0)
            colv = np.full(int(kb[b]) * 128, -1.0, np.float32)
            colv[:k] = (dl % 128).astype(np.float32)
            dcol[:, int(k_off[b]):int(k_end[b])] = colv.reshape(-1, 128).T
        sl = slice(c * S, (c + 1) * S)
        xs = np.asarray(x[:, sl, :], np.float32)          # [2, S, 128]
        xr = np.concatenate([xs[0], xs[1]], axis=1)       # [S, 256] fused rows
        xrt = np.ascontiguousarray(
            xr.reshape(SI, 128, 256).transpose(1, 0, 2))  # [128, SI, 256]
        in_maps.append({
            "gidx": _wrap16(g16),
            "we": np.ascontiguousarray(nwe.reshape(-1, 128).T.astype(bf)),
            "wo": np.ascontiguousarray(nwo.reshape(-1, 128).T.astype(bf)),
            "dstcol": np.ascontiguousarray(dcol),
            "xT": np.ascontiguousarray(xs.transpose(0, 2, 1)),   # [2, 128, S] f32
            "xrt": np.ascontiguousarray(xrt.astype(bf)),          # bf16
        })

    iota = np.ascontiguousarray(
        np.broadcast_to(np.arange(128, dtype=np.float32), (128, 128)))
    for m in in_maps:
        m["iota"] = iota

    meta = {"L2g": L2g, "NCALL": NCALL, "NCH": NCH, "NCHP": NCHP, "blocks": blocks}
    return in_maps, meta


def _pack_weights(W1, W2, W3, g1, be1, g2, be2, g3, be3):
    import ml_dtypes
    bf = ml_dtypes.bfloat16
    W1 = np.asarray(W1, np.float32)
    W2 = np.asarray(W2, np.float32)
    W3 = np.asarray(W3, np.float32)
    w1cat = np.concatenate([W1[0] - W1[2], W1[1], W1[2]], axis=1)  # [128, 96] f32

    def fuse(w):  # [ci, co] -> [2ci, 2co] block-diag over batch
        ci, co = w.shape
        out = np.zeros((2 * ci, 2 * co), np.float32)
        out[:ci, :co] = w
        out[ci:, co:] = w
        return out

    w2bundle = np.concatenate([fuse(W2[0] - W2[2]), fuse(W2[1]), fuse(W2[2])], axis=1)
    return {
        "w1cat": np.ascontiguousarray(w1cat),
        "w2bundle": np.ascontiguousarray(w2bundle.astype(bf)),      # [64, 192]
        "w3a": np.ascontiguousarray(fuse(W3[0] - W3[2]).astype(bf)),  # [64, 256]
        "w3b": np.ascontiguousarray(fuse(W3[1]).astype(bf)),
        "w3c": np.ascontiguousarray(fuse(2.0 * W3[2]).astype(bf)),
        "g1": np.asarray(g1, np.float32)[None, :], "be1": np.asarray(be1, np.float32)[None, :],
        "g2": np.asarray(g2, np.float32)[None, :], "be2": np.asarray(be2, np.float32)[None, :],
        "g3": np.asarray(g3, np.float32)[None, :], "be3": np.asarray(be3, np.float32)[None, :],
    }


def _build_program(meta, debug=False):
    import concourse.bacc as bacc
    import concourse.mybir as mybir
    import concourse.tile as tile
    from concourse.library_config import mlp
    from concourse.masks import make_identity

    f32 = mybir.dt.float32
    bf16 = mybir.dt.bfloat16
    fp8 = mybir.dt.float8e4
    i16 = mybir.dt.int16
    AT = mybir.AluOpType
    stat_dt = fp8 if STAT_FP8 else bf16
    L2g, NCALL, NCH, NCHP, blocks = (
        meta["L2g"], meta["NCALL"], meta["NCH"], meta["NCHP"], meta["blocks"])

    nc = bacc.Bacc("TRN2", target_bir_lowering=False, debug=False, num_devices=NC,
                   num_swdge_queues=NQ,
                   dynamic_dma_scratch_size=int(os.environ.get("BK_SCRATCH", "16384")))

    # ---- I/O ----
    gidx = nc.dram_tensor("gidx", [128, L2g // 16], i16, kind="ExternalInput")
    we_d = nc.dram_tensor("we", [128, NCHP], bf16, kind="ExternalInput")
    wo_d = nc.dram_tensor("wo", [128, NCHP], bf16, kind="ExternalInput")
    dstcol_d = nc.dram_tensor("dstcol", [128, NCHP], f32, kind="ExternalInput")
    iota_d = nc.dram_tensor("iota", [128, 128], f32, kind="ExternalInput")
    xT = nc.dram_tensor("xT", [B, 128, S], f32, kind="ExternalInput")
    xrt = nc.dram_tensor("xrt", [128, SI, 256], bf16, kind="ExternalInput")
    w1cat = nc.dram_tensor("w1cat", [128, 96], f32, kind="ExternalInput")
    w2bundle = nc.dram_tensor("w2bundle", [64, 192], bf16, kind="ExternalInput")
    w3a_d = nc.dram_tensor("w3a", [64, 256], bf16, kind="ExternalInput")
    w3b_d = nc.dram_tensor("w3b", [64, 256], bf16, kind="ExternalInput")
    w3c_d = nc.dram_tensor("w3c", [64, 256], bf16, kind="ExternalInput")
    gbe_w = {"g1": 32, "be1": 32, "g2": 32, "be2": 32, "g3": 128, "be3": 128}
    gbe = {nm: nc.dram_tensor(nm, [1, w], f32, kind="ExternalInput") for nm, w in gbe_w.items()}
    out_d = nc.dram_tensor("out", [128, SI, 256], f32, kind="ExternalOutput")

    # ---- internal DRAM ----
    full = [nc.dram_tensor(f"full{i}", [N, 64], bf16, addr_space="Shared") for i in range(6)]
    shard = [nc.dram_tensor(f"shard{i}", [S, 64], bf16) for i in range(6)]
    st_in = [nc.dram_tensor(f"stin{i}", [1, 512], f32) for i in range(3)]
    st_out = [nc.dram_tensor(f"stout{i}", [1, 512], f32, addr_space="Shared") for i in range(3)]
    o3d = nc.dram_tensor("o3d", [128, SI, 256], bf16)

    RG = [list(range(NC))]

    def shard_tile_ap(i):
        return shard[i][:].rearrange("(p i) e -> p i e", p=128)

    with tile.TileContext(nc) as tc, contextlib.ExitStack() as ctx:
        const = ctx.enter_context(tc.tile_pool(name="const", bufs=1))
        sb = ctx.enter_context(tc.tile_pool(name="sb", bufs=1))
        gp = ctx.enter_context(tc.tile_pool(name="gp", bufs=2))
        rp = ctx.enter_context(tc.tile_pool(name="rp", bufs=3))
        tp2 = ctx.enter_context(tc.tile_pool(name="tp2", bufs=2))
        og = ctx.enter_context(tc.tile_pool(name="og", bufs=2))
        pp = ctx.enter_context(tc.tile_pool(name="pp", bufs=3, space="PSUM"))
        pd = ctx.enter_context(tc.tile_pool(name="pd", bufs=2, space="PSUM"))
        pp1 = ctx.enter_context(tc.tile_pool(name="pp1", bufs=1, space="PSUM"))

        nc.gpsimd.load_library(mlp)

        ident = const.tile([128, 128], bf16)
        make_identity(nc, ident[:])
        ones_k = const.tile([128, 1], f32)
        nc.vector.memset(ones_k[:], 1.0)
        ones_m = const.tile([1, 128], f32)
        nc.vector.memset(ones_m[:], 1.0)

        gidx_sb = const.tile([128, L2g // 16], i16)
        we_sb = const.tile([128, NCHP], bf16)
        wo_sb = const.tile([128, NCHP], bf16)
        dcol_sb = const.tile([128, NCHP], f32)
        iota_sb = const.tile([128, 128], f32)
        nc.sync.dma_start(gidx_sb[:], gidx[:])
        nc.sync.dma_start(we_sb[:], we_d[:])
        nc.sync.dma_start(wo_sb[:], wo_d[:])
        nc.sync.dma_start(dcol_sb[:], dstcol_d[:])
        nc.sync.dma_start(iota_sb[:], iota_d[:])

        w1_sb = const.tile([128, 96], f32)
        w2_sb = const.tile([64, 192], bf16)
        w3a = const.tile([64, 256], bf16)
        w3b = const.tile([64, 256], bf16)
        w3c = const.tile([64, 256], bf16)
        nc.sync.dma_start(w1_sb[:], w1cat[:])
        nc.sync.dma_start(w2_sb[:], w2bundle[:])
        nc.sync.dma_start(w3a[:], w3a_d[:])
        nc.sync.dma_start(w3b[:], w3b_d[:])
        nc.sync.dma_start(w3c[:], w3c_d[:])
        gbe_sb = {}
        for nm, w in gbe_w.items():
            t = const.tile([1, w], f32)
            nc.sync.dma_start(t[:], gbe[nm][:])
            gbe_sb[nm] = t

        # ---- one-hot stationaries: built once, SBUF-resident ----
        stat = const.tile([128, NCH, 128], stat_dt)
        for k in range(NCH):
            nc.vector.tensor_scalar(
                out=stat[:, k, :], in0=iota_sb[:], scalar1=dcol_sb[:, k:k + 1],
                scalar2=None, op0=AT.is_equal)

        # ---- propagation ----
        def prop(t_i, prows):
            """prows: bf16 [128, SI, 64] destination rows (node-major)."""
            t2 = full[t_i][:].rearrange("(a b) e -> a (b e)", b=2)  # [N/2, 128] bf16
            Rs = []
            for w in range(NCALL):
                G = gp.tile([128, GC, 128], bf16, tag="G")
                nc.gpsimd.dma_gather(G[:], t2,
                                     gidx_sb[:, w * (GCALL // 16):(w + 1) * (GCALL // 16)],
                                     GCALL, GCALL, 128, queue_num=w % NQ)
                ws = slice(w * GC, (w + 1) * GC)
                T = tp2.tile([128, GC, 64], bf16, tag="T")
                nc.vector.tensor_tensor(
                    out=T[:], in0=G[:, :, 64:128],
                    in1=wo_sb[:, ws, None].to_broadcast([128, GC, 64]), op=AT.mult)
                R = rp.tile([128, GC, 64], bf16, tag="R")
                nc.vector.tensor_tensor(
                    out=R[:], in0=G[:, :, 0:64],
                    in1=we_sb[:, ws, None].to_broadcast([128, GC, 64]), op=AT.mult)
                nc.vector.tensor_tensor(out=R[:], in0=R[:], in1=T[:], op=AT.add)
                Rs.append(R)
            for b, (k0, k1) in enumerate(blocks):
                ps = pp.tile([128, 64], f32, tag="red")
                for k in range(k0, k1):
                    nc.tensor.matmul(ps[:], lhsT=stat[:, k, :],
                                     rhs=Rs[k // GC][:, k % GC, :],
                                     start=(k == k0), stop=(k == k1 - 1))
                nc.scalar.copy(out=prows[:, b, :], in_=ps[:])

        # ---- BatchNorm helpers ----
        def bn_coeffs(sums, cmid, g_t, be_t, st_i):
            F = 2 * cmid
            ps = pp1.tile([1, 512], f32, tag="bnps")
            nc.tensor.matmul(ps[:, 0:2 * F], lhsT=ones_k[:], rhs=sums[:, 0:2 * F],
                             start=True, stop=True)
            stt = sb.tile([1, 512], f32, tag="bnstt")
            nc.vector.tensor_copy(out=stt[:, 0:2 * F], in_=ps[:, 0:2 * F])
            if 2 * F < 512:
                nc.vector.memset(stt[:, 2 * F:], 0.0)
            nc.sync.dma_start(st_in[st_i][:], stt[:])
            nc.gpsimd.collective_compute(
                "AllReduce", AT.add, replica_groups=RG,
                ins=[st_in[st_i][:].opt()], outs=[st_out[st_i][:].opt()])
            stf = sb.tile([1, 512], f32, tag="bnstf")
            nc.sync.dma_start(stf[:], st_out[st_i][:])
            cs = sb.tile([1, 8 * cmid], f32, tag="bncs")
            nc.vector.tensor_tensor(out=cs[:, 0:cmid], in0=stf[:, 0:cmid],
                                    in1=stf[:, cmid:F], op=AT.add)
            nc.vector.tensor_tensor(out=cs[:, cmid:2 * cmid], in0=stf[:, F:F + cmid],
                                    in1=stf[:, F + cmid:2 * F], op=AT.add)
            inv_n = 1.0 / float(B * N)
            mu = cs[:, 4 * cmid:5 * cmid]
            nc.vector.tensor_scalar_mul(mu, cs[:, 0:cmid], inv_n)
            msq = cs[:, 5 * cmid:6 * cmid]
            nc.vector.tensor_scalar_mul(msq, cs[:, cmid:2 * cmid], inv_n)
            var = cs[:, 6 * cmid:7 * cmid]
            nc.vector.tensor_tensor(out=var, in0=mu, in1=mu, op=AT.mult)
            nc.vector.tensor_tensor(out=var, in0=msq, in1=var, op=AT.subtract)
            nc.vector.tensor_scalar_add(var, var, EPS)
            std = cs[:, 7 * cmid:8 * cmid]
            nc.scalar.sqrt(std, var)
            rstd = cs[:, 6 * cmid:7 * cmid]
            nc.vector.reciprocal(rstd, std)
            s_ = cs[:, 2 * cmid:3 * cmid]
            nc.vector.tensor_tensor(out=s_, in0=g_t[:], in1=rstd, op=AT.mult)
            o_ = cs[:, 3 * cmid:4 * cmid]
            nc.vector.tensor_tensor(out=o_, in0=mu, in1=s_, op=AT.mult)
            nc.vector.tensor_tensor(out=o_, in0=be_t[:], in1=o_, op=AT.subtract)
            sf = sb.tile([1, 512], f32, tag="bnsf")
            nc.vector.tensor_copy(out=sf[:, 0:cmid], in_=s_)
            nc.vector.tensor_copy(out=sf[:, cmid:F], in_=s_)
            nc.vector.tensor_copy(out=sf[:, F:F + cmid], in_=o_)
            nc.vector.tensor_copy(out=sf[:, F + cmid:2 * F], in_=o_)
            psb = pp1.tile([128, 512], f32, tag="bnps")
            nc.tensor.matmul(psb[:, 0:2 * F], lhsT=ones_m[:], rhs=sf[:, 0:2 * F],
                             start=True, stop=True)
            rep = sb.tile([128, 512], f32, tag="bnrep")
            nc.vector.tensor_copy(out=rep[:, 0:2 * F], in_=psb[:, 0:2 * F])
            return rep

        def bn_relu_rows(orows, cmid, g_t, be_t, st_i):
            """orows bf16 [128, SI, F] -> z bf16 same shape (tag Z)."""
            F = 2 * cmid
            sums = sb.tile([128, 512], f32, tag="bnsums")
            nc.vector.tensor_reduce(out=sums[:, 0:F],
                                    in_=orows[:].rearrange("p i c -> p c i"),
                                    axis=mybir.AxisListType.X, op=AT.add)
            nc.vector.memset(sums[:, F:2 * F], 0.0)
            for gq in range(SI // 8):
                sq = tp2.tile([128, 8, F], bf16, tag="bnsq")
                nc.vector.tensor_tensor(out=sq[:], in0=orows[:, gq * 8:(gq + 1) * 8, :],
                                        in1=orows[:, gq * 8:(gq + 1) * 8, :], op=AT.mult)
                red2 = tp2.tile([128, F], f32, tag="bnred2")
                nc.vector.tensor_reduce(out=red2[:], in_=sq[:].rearrange("p i c -> p c i"),
                                        axis=mybir.AxisListType.X, op=AT.add)
                nc.vector.tensor_tensor(out=sums[:, F:2 * F], in0=sums[:, F:2 * F],
                                        in1=red2[:], op=AT.add)
            rep = bn_coeffs(sums, cmid, g_t, be_t, st_i)
            zr = sb.tile([128, SI, F], bf16, tag="Z")
            nc.vector.tensor_tensor(out=zr[:], in0=orows[:],
                                    in1=rep[:, None, 0:F].to_broadcast([128, SI, F]), op=AT.mult)
            nc.vector.tensor_tensor(out=zr[:], in0=zr[:],
                                    in1=rep[:, None, F:2 * F].to_broadcast([128, SI, F]), op=AT.add)
            nc.vector.tensor_scalar_max(zr[:], zr[:], 0.0)
            return zr

        # ================= Layer 1 dense (f32 in, bf16 out) =================
        # hold1 [128, SI, 2, 96]: per (tile, batch) psum [a32 u32 v32]
        hold1 = sb.tile([128, SI, 2, 96], bf16, tag="hold")
        for g in range(SI // 8):
            for b in range(B):
                xtb = og.tile([128, 1024], f32, tag="xtb")
                nc.sync.dma_start(xtb[:], xT[b, :, g * 1024:(g + 1) * 1024])
                for j in range(8):
                    i = g * 8 + j
                    psd = pd.tile([128, 256], f32, tag="dps")
                    nc.tensor.matmul(psd[:, 0:96], lhsT=xtb[:, j * 128:(j + 1) * 128],
                                     rhs=w1_sb[:], start=True, stop=True)
                    nc.scalar.copy(out=hold1[:, i, b, :], in_=psd[:, 0:96])
        # v1 table -> shard0 (per batch, inner 32 contiguous)
        for b in range(B):
            nc.sync.dma_start(shard_tile_ap(0)[:, :, b * 32:(b + 1) * 32],
                              hold1[:, :, b, 64:96])
        nc.gpsimd.collective_compute("AllGather", AT.bypass, replica_groups=RG,
                                     ins=[shard[0][:].opt()], outs=[full[0][:].opt()])
        p11 = sb.tile([128, SI, 64], bf16, tag="P")
        prop(0, p11)
        # q1 = u1 + 2*p11  (u1 = hold1[:, :, :, 32:64] viewed as fused 64)
        q1 = sb.tile([128, SI, 64], bf16, tag="Q")
        nc.vector.scalar_tensor_tensor(
            out=q1[:].rearrange("p i (b c) -> p i b c", b=2), in0=p11[:].rearrange("p i (b c) -> p i b c", b=2),
            scalar=2.0, in1=hold1[:, :, :, 32:64], op0=AT.mult, op1=AT.add)
        nc.sync.dma_start(shard_tile_ap(1), q1[:])
        nc.gpsimd.collective_compute("AllGather", AT.bypass, replica_groups=RG,
                                     ins=[shard[1][:].opt()], outs=[full[1][:].opt()])
        p12 = sb.tile([128, SI, 64], bf16, tag="P")
        prop(1, p12)
        o1 = sb.tile([128, SI, 64], bf16, tag="O")
        nc.vector.tensor_tensor(
            out=o1[:].rearrange("p i (b c) -> p i b c", b=2),
            in0=p12[:].rearrange("p i (b c) -> p i b c", b=2),
            in1=hold1[:, :, :, 0:32], op=AT.add)
        z2 = bn_relu_rows(o1, C_MID, gbe_sb["g1"], gbe_sb["be1"], 0)

        # ================= Layer 2 (bf16) =================
        hold2 = sb.tile([128, SI, 192], bf16, tag="hold")
        for i in range(SI):
            tp = pd.tile([64, 128], bf16, tag="tps")
            nc.tensor.transpose(out=tp[:], in_=z2[:, i, :], identity=ident[:])
            ztc = tp2.tile([64, 128], bf16, tag="ztc")
            nc.scalar.copy(out=ztc[:], in_=tp[:])
            psd = pd.tile([128, 256], f32, tag="dps")
            nc.tensor.matmul(psd[:, 0:192], lhsT=ztc[:], rhs=w2_sb[:], start=True, stop=True)
            nc.scalar.copy(out=hold2[:, i, :], in_=psd[:, 0:192])
        nc.sync.dma_start(shard_tile_ap(2), hold2[:, :, 128:192])
        nc.gpsimd.collective_compute("AllGather", AT.bypass, replica_groups=RG,
                                     ins=[shard[2][:].opt()], outs=[full[2][:].opt()])
        p21 = sb.tile([128, SI, 64], bf16, tag="P")
        prop(2, p21)
        q2 = sb.tile([128, SI, 64], bf16, tag="Q")
        nc.vector.scalar_tensor_tensor(out=q2[:], in0=p21[:], scalar=2.0,
                                       in1=hold2[:, :, 64:128], op0=AT.mult, op1=AT.add)
        nc.sync.dma_start(shard_tile_ap(3), q2[:])
        nc.gpsimd.collective_compute("AllGather", AT.bypass, replica_groups=RG,
                                     ins=[shard[3][:].opt()], outs=[full[3][:].opt()])
        p22 = sb.tile([128, SI, 64], bf16, tag="P")
        prop(3, p22)
        o2 = sb.tile([128, SI, 64], bf16, tag="O")
        nc.vector.tensor_tensor(out=o2[:], in0=p22[:], in1=hold2[:, :, 0:64], op=AT.add)
        z3 = bn_relu_rows(o2, C_MID, gbe_sb["g2"], gbe_sb["be2"], 1)

        # ================= Layer 3 =================
        nc.sync.dma_start(shard_tile_ap(4), z3[:])
        nc.gpsimd.collective_compute("AllGather", AT.bypass, replica_groups=RG,
                                     ins=[shard[4][:].opt()], outs=[full[4][:].opt()])
        t1r = sb.tile([128, SI, 64], bf16, tag="P")
        prop(4, t1r)
        nc.sync.dma_start(shard_tile_ap(5), t1r[:])
        nc.gpsimd.collective_compute("AllGather", AT.bypass, replica_groups=RG,
                                     ins=[shard[5][:].opt()], outs=[full[5][:].opt()])
        p32 = sb.tile([128, SI, 64], bf16, tag="Q")
        prop(5, p32)

        acc_s = sb.tile([128, 512], f32, tag="bnsums")
        nc.vector.memset(acc_s[:], 0.0)
        for g in range(SI // 8):
            hold3 = og.tile([128, 8, 256], bf16, tag="o3g")
            for j in range(8):
                i = g * 8 + j
                psd = pd.tile([128, 256], f32, tag="dps")
                for (rows_t, w_t, st_, sp_) in ((z3, w3a, True, False),
                                                (t1r, w3b, False, False),
                                                (p32, w3c, False, True)):
                    tp = pd.tile([64, 128], bf16, tag="tps")
                    nc.tensor.transpose(out=tp[:], in_=rows_t[:, i, :], identity=ident[:])
                    ztc = tp2.tile([64, 128], bf16, tag="ztc")
                    nc.scalar.copy(out=ztc[:], in_=tp[:])
                    nc.tensor.matmul(psd[:], lhsT=ztc[:], rhs=w_t[:], start=st_, stop=sp_)
                nc.scalar.copy(out=hold3[:, j, :], in_=psd[:])
            nc.sync.dma_start(o3d[:, g * 8:(g + 1) * 8, :], hold3[:])
            red = sb.tile([128, 512], f32, tag="bnred")
            nc.vector.tensor_reduce(out=red[:, 0:256], in_=hold3[:].rearrange("p j c -> p c j"),
                                    axis=mybir.AxisListType.X, op=AT.add)
            nc.vector.memset(red[:, 256:512], 0.0)
            for h in range(2):
                sqh = og.tile([128, 4, 256], bf16, tag="o3sq")
                nc.vector.tensor_tensor(out=sqh[:], in0=hold3[:, h * 4:(h + 1) * 4, :],
                                        in1=hold3[:, h * 4:(h + 1) * 4, :], op=AT.mult)
                red2 = sb.tile([128, 256], f32, tag="bnred2b")
                nc.vector.tensor_reduce(out=red2[:], in_=sqh[:].rearrange("p j c -> p c j"),
                                        axis=mybir.AxisListType.X, op=AT.add)
                nc.vector.tensor_tensor(out=red[:, 256:512], in0=red[:, 256:512],
                                        in1=red2[:], op=AT.add)
            nc.vector.tensor_tensor(out=acc_s[:], in0=acc_s[:], in1=red[:], op=AT.add)
        rep3 = bn_coeffs(acc_s, C_OUT, gbe_sb["g3"], gbe_sb["be3"], 2)

        for g in range(SI // 4):
            gs = slice(g * 4, (g + 1) * 4)
            o3c = og.tile([128, 4, 256], bf16, tag="o3c")
            nc.sync.dma_start(o3c[:], o3d[:, gs, :])
            zc = og.tile([128, 4, 256], f32, tag="zc")
            nc.vector.tensor_tensor(out=zc[:], in0=o3c[:],
                                    in1=rep3[:, None, 0:256].to_broadcast([128, 4, 256]),
                                    op=AT.mult)
            nc.vector.tensor_tensor(out=zc[:], in0=zc[:],
                                    in1=rep3[:, None, 256:512].to_broadcast([128, 4, 256]),
                                    op=AT.add)
            nc.vector.tensor_scalar_max(zc[:], zc[:], 0.0)
            xc = og.tile([128, 4, 256], bf16, tag="xc")
            nc.sync.dma_start(xc[:], xrt[:, gs, :])
            nc.vector.tensor_tensor(out=zc[:], in0=zc[:], in1=xc[:], op=AT.add)
            nc.vector.tensor_scalar_max(zc[:], zc[:], 0.0)
            nc.sync.dma_start(out_d[:, gs, :], zc[:])

    nc.compile()
    return nc


def kernel(x, edge_index, edge_weight,
           W1, b1, g1, be1, W2, b2, g2, be2, W3, b3, g3, be3):
    from concourse.bass_utils import run_bass_kernel_spmd

    x = np.asarray(x, np.float32)
    in_maps, meta = _host_prep(x, edge_index, edge_weight)
    wts = _pack_weights(W1, W2, W3, g1, be1, g2, be2, g3, be3)
    for m in in_maps:
        m.update(wts)

    key = (meta["L2g"], meta["NCH"], tuple(k for _, k in meta["blocks"]))
    if key not in _CACHE:
        _CACHE[key] = _build_program(meta)
    nc = _CACHE[key]

    trace = os.environ.get("BK_TRACE", "0") == "1"
    kw = {"trace": True} if trace else {}
    res = run_bass_kernel_spmd(nc, in_maps, list(range(NC)), **kw)
    if trace:
        print(f"HW exec time: {res.exec_time_ns} ns (mean {res.mean_exec_time_ns})")

    out = np.empty((B, N, 128), np.float32)
    for c in range(NC):
        oc = res.results[c]["out"]  # [128, SI, 256] tile layout
        rows = oc.transpose(1, 0, 2).reshape(S, 256)  # node = i*128 + p
        out[0, c * S:(c + 1) * S, :] = rows[:, 0:128]
        out[1, c * S:(c + 1) * S, :] = rows[:, 128:256]
    kernel._last_results = res
    return out


# revision 7
# speedup vs baseline: 1.7931x; 1.6961x over previous
"""Trainium2 Bass kernel for nn_BottleneckBlock (Chebyshev GNN bottleneck block).

v3: bf16 tables/gathers, fp8 SBUF-resident one-hot stationaries,
v-first dense passes (AllGather starts before the a/u pass),
BN stats folded into the propagation window (per-block-group),
o3 kept in SBUF (no DRAM round-trip), deeper gather buffering.

Math restructure (per Chebyshev layer, K=3) as baseline:
    out = x (W0 - W2) + L(x W1 + 2 L (x W2))  for layers 1, 2
    layer 3 standard recursion.  Biases before BatchNorm cancel.
Sharding: nodes split 8 ways; per-prop AllGather of the bf16 table in
permuted row order (shard writes contiguous); dma_gather of paired
256B rows; TensorE one-hot reduction per 128-dst block.
"""

import os
import contextlib
import numpy as np

NC = 8
N = 49152
B = 2
C_MID = 32
C_OUT = 128
EPS = 1e-5
S = N // NC           # 6144 nodes per core
SI = S // 128         # 48 dst blocks
GCALL = int(os.environ.get("BK_GCALL", "1024"))
GC = GCALL // 128     # chunks per gather call
NQ = int(os.environ.get("BK_NQ", "4"))
STAT_FP8 = os.environ.get("BK_STATFP8", "1") == "1"

_CACHE = {}


def _wrap16(idx):
    a = np.asarray(idx, np.int16).reshape(-1, 16).T
    return np.ascontiguousarray(np.tile(a, (8, 1)))


def _perm_row(node):
    """Global node id -> permuted table row (per-core block-interleaved)."""
    c = node // S
    nl = node % S
    return c * S + (nl % 128) * SI + nl // 128


def _host_prep(x, edge_index, edge_weight):
    import ml_dtypes
    bf = ml_dtypes.bfloat16

    src = np.asarray(edge_index[0], np.int64)
    dst = np.asarray(edge_index[1], np.int64)
    ew = np.asarray(edge_weight, np.float32)

    deg = np.bincount(src, weights=ew.astype(np.float64), minlength=N).astype(np.float32)
    dinv = np.where(deg > 0, 1.0 / np.sqrt(np.maximum(deg, 1e-30)), 0.0).astype(np.float32)
    nw = (-dinv[src] * ew * dinv[dst]).astype(np.float32)

    per_core = []
    for c in range(NC):
        sel = np.nonzero((dst >= c * S) & (dst < (c + 1) * S))[0]
        d_loc = (dst[sel] - c * S).astype(np.int64)
        order = np.argsort(d_loc // 128, kind="stable")
        per_core.append((sel[order], d_loc[order]))

    kb = np.zeros(SI, np.int64)  # chunks per block (unified across cores)
    for c in range(NC):
        _, d_loc = per_core[c]
        cnt = np.bincount(d_loc // 128, minlength=SI)
        kb = np.maximum(kb, -(-cnt // 128))
    kb = np.maximum(kb, 1)
    k_end = np.cumsum(kb)
    k_off = k_end - kb
    NCH = int(k_end[-1])
    blocks = [(int(k_off[b]), int(k_end[b])) for b in range(SI)]
    L2 = NCH * 128
    L2g = -(-L2 // GCALL) * GCALL
    NCALL = L2g // GCALL
    NCHP = L2g // 128    # padded chunk count (we/wo sized to this)

    in_maps = []
    for c in range(NC):
        sel, d_loc = per_core[c]
        g16 = np.zeros(L2g, np.int16)
        nwe = np.zeros(L2g, np.float32)
        nwo = np.zeros(L2g, np.float32)
        dcol = np.full((128, NCHP), -1.0, np.float32)
        cnt = np.bincount(d_loc // 128, minlength=SI)
        eo = np.concatenate([[0], np.cumsum(cnt)])
        for b in range(SI):
            e_ids = sel[eo[b]:eo[b + 1]]
            dl = d_loc[eo[b]:eo[b + 1]]
            o = int(k_off[b]) * 128
            k = e_ids.size
            rowp = _perm_row(src[e_ids])
            g16[o:o + k] = (rowp >> 1).astype(np.int16)
            par = (rowp & 1).astype(bool)
            w = nw[e_ids]
            nwe[o:o + k] = np.where(~par, w, 0.0)
            nwo[o:o + k] = np.where(par, w, 0.0)
            colv = np.full(int(kb[b]) * 128, -1.0, np.float32)
            colv[:k] = (dl % 128).astype(np.float32)
            dcol[:, int(k_off[b]):int(k_end[b])] = colv.reshape(-1, 128).T
        sl = slice(c * S, (c + 1) * S)
        xs = np.asarray(x[:, sl, :], np.float32)          # [2, S, 128]
        xr = np.concatenate([xs[0], xs[1]], axis=1)       # [S, 256] fused rows
        xrt = np.ascontiguousarray(
            xr.reshape(SI, 128, 256).transpose(1, 0, 2))  # [128, SI, 256]
        in_maps.append({
            "gidx": _wrap16(g16),
            "we": np.ascontiguousarray(nwe.reshape(-1, 128).T.astype(bf)),
            "wo": np.ascontiguousarray(nwo.reshape(-1, 128).T.astype(bf)),
            "dstcol": np.ascontiguousarray(dcol),
            "xT": np.ascontiguousarray(xs.transpose(0, 2, 1)),   # [2, 128, S] f32
            "xrt": np.ascontiguousarray(xrt.astype(bf)),          # bf16
        })

    iota = np.ascontiguousarray(
        np.broadcast_to(np.arange(128, dtype=np.float32), (128, 128)))
    for m in in_maps:
        m["iota"] = iota

    meta = {"L2g": L2g, "NCALL": NCALL, "NCH": NCH, "NCHP": NCHP, "blocks": blocks}
    return in_maps, meta


def _pack_weights(W1, W2, W3, g1, be1, g2, be2, g3, be3):
    import ml_dtypes
    bf = ml_dtypes.bfloat16
    W1 = np.asarray(W1, np.float32)
    W2 = np.asarray(W2, np.float32)
    W3 = np.asarray(W3, np.float32)
    # layer1: [a|u|v] = [W0-W2 | W1 | W2]
    w1cat = np.concatenate([W1[0] - W1[2], W1[1], W1[2]], axis=1)  # [128, 96] f32

    def fuse(w):  # [ci, co] -> [2ci, 2co] block-diag over batch
        ci, co = w.shape
        out = np.zeros((2 * ci, 2 * co), np.float32)
        out[:ci, :co] = w
        out[ci:, co:] = w
        return out

    w2bundle = np.concatenate([fuse(W2[0] - W2[2]), fuse(W2[1]), fuse(W2[2])], axis=1)
    return {
        "w1cat": np.ascontiguousarray(w1cat),
        "w2bundle": np.ascontiguousarray(w2bundle.astype(bf)),      # [64, 192]
        "w3a": np.ascontiguousarray(fuse(W3[0] - W3[2]).astype(bf)),  # [64, 256]
        "w3b": np.ascontiguousarray(fuse(W3[1]).astype(bf)),
        "w3c": np.ascontiguousarray(fuse(2.0 * W3[2]).astype(bf)),
        "g1": np.asarray(g1, np.float32)[None, :], "be1": np.asarray(be1, np.float32)[None, :],
        "g2": np.asarray(g2, np.float32)[None, :], "be2": np.asarray(be2, np.float32)[None, :],
        "g3": np.asarray(g3, np.float32)[None, :], "be3": np.asarray(be3, np.float32)[None, :],
    }


def _build_program(meta, debug=False):
    import concourse.bacc as bacc
    import concourse.mybir as mybir
    import concourse.tile as tile
    from concourse.library_config import mlp
    from concourse.masks import make_identity

    f32 = mybir.dt.float32
    bf16 = mybir.dt.bfloat16
    fp8 = mybir.dt.float8e4
    i16 = mybir.dt.int16
    AT = mybir.AluOpType
    stat_dt = fp8 if STAT_FP8 else bf16
    L2g, NCALL, NCH, NCHP, blocks = (
        meta["L2g"], meta["NCALL"], meta["NCH"], meta["NCHP"], meta["blocks"])

    nc = bacc.Bacc("TRN2", target_bir_lowering=False, debug=False, num_devices=NC,
                   num_swdge_queues=NQ,
                   dynamic_dma_scratch_size=int(os.environ.get("BK_SCRATCH", "16384")))

    # ---- I/O ----
    gidx = nc.dram_tensor("gidx", [128, L2g // 16], i16, kind="ExternalInput")
    we_d = nc.dram_tensor("we", [128, NCHP], bf16, kind="ExternalInput")
    wo_d = nc.dram_tensor("wo", [128, NCHP], bf16, kind="ExternalInput")
    dstcol_d = nc.dram_tensor("dstcol", [128, NCHP], f32, kind="ExternalInput")
    iota_d = nc.dram_tensor("iota", [128, 128], f32, kind="ExternalInput")
    xT = nc.dram_tensor("xT", [B, 128, S], f32, kind="ExternalInput")
    xrt = nc.dram_tensor("xrt", [128, SI, 256], bf16, kind="ExternalInput")
    w1cat = nc.dram_tensor("w1cat", [128, 96], f32, kind="ExternalInput")
    w2bundle = nc.dram_tensor("w2bundle", [64, 192], bf16, kind="ExternalInput")
    w3a_d = nc.dram_tensor("w3a", [64, 256], bf16, kind="ExternalInput")
    w3b_d = nc.dram_tensor("w3b", [64, 256], bf16, kind="ExternalInput")
    w3c_d = nc.dram_tensor("w3c", [64, 256], bf16, kind="ExternalInput")
    gbe_w = {"g1": 32, "be1": 32, "g2": 32, "be2": 32, "g3": 128, "be3": 128}
    gbe = {nm: nc.dram_tensor(nm, [1, w], f32, kind="ExternalInput") for nm, w in gbe_w.items()}
    out_d = nc.dram_tensor("out", [128, SI, 256], f32, kind="ExternalOutput")

    # ---- internal DRAM ----
    full = [nc.dram_tensor(f"full{i}", [N, 64], bf16, addr_space="Shared") for i in range(6)]
    shard = [nc.dram_tensor(f"shard{i}", [S, 64], bf16) for i in range(6)]
    st_in = [nc.dram_tensor(f"stin{i}", [1, 512], f32) for i in range(3)]
    st_out = [nc.dram_tensor(f"stout{i}", [1, 512], f32, addr_space="Shared") for i in range(3)]

    RG = [list(range(NC))]

    def shard_tile_ap(i):
        return shard[i][:].rearrange("(p i) e -> p i e", p=128)

    with tile.TileContext(nc) as tc, contextlib.ExitStack() as ctx:
        const = ctx.enter_context(tc.tile_pool(name="const", bufs=1))
        sb = ctx.enter_context(tc.tile_pool(name="sb", bufs=1))
        gp = ctx.enter_context(tc.tile_pool(name="gp", bufs=4))
        rp = ctx.enter_context(tc.tile_pool(name="rp", bufs=4))
        tp2 = ctx.enter_context(tc.tile_pool(name="tp2", bufs=2))
        og = ctx.enter_context(tc.tile_pool(name="og", bufs=2))
        pp = ctx.enter_context(tc.tile_pool(name="pp", bufs=3, space="PSUM"))
        pd = ctx.enter_context(tc.tile_pool(name="pd", bufs=2, space="PSUM"))
        pp1 = ctx.enter_context(tc.tile_pool(name="pp1", bufs=1, space="PSUM"))

        nc.gpsimd.load_library(mlp)

        ident = const.tile([128, 128], bf16)
        make_identity(nc, ident[:])
        ones_k = const.tile([128, 1], f32)
        nc.vector.memset(ones_k[:], 1.0)
        ones_m = const.tile([1, 128], f32)
        nc.vector.memset(ones_m[:], 1.0)

        gidx_sb = const.tile([128, L2g // 16], i16)
        we_sb = const.tile([128, NCHP], bf16)
        wo_sb = const.tile([128, NCHP], bf16)
        dcol_sb = const.tile([128, NCHP], f32)
        iota_sb = const.tile([128, 128], f32)
        nc.sync.dma_start(gidx_sb[:], gidx[:])
        nc.sync.dma_start(we_sb[:], we_d[:])
        nc.sync.dma_start(wo_sb[:], wo_d[:])
        nc.sync.dma_start(dcol_sb[:], dstcol_d[:])
        nc.sync.dma_start(iota_sb[:], iota_d[:])

        w1_sb = const.tile([128, 96], f32)
        w2_sb = const.tile([64, 192], bf16)
        w3a = const.tile([64, 256], bf16)
        w3b = const.tile([64, 256], bf16)
        w3c = const.tile([64, 256], bf16)
        nc.sync.dma_start(w1_sb[:], w1cat[:])
        nc.sync.dma_start(w2_sb[:], w2bundle[:])
        nc.sync.dma_start(w3a[:], w3a_d[:])
        nc.sync.dma_start(w3b[:], w3b_d[:])
        nc.sync.dma_start(w3c[:], w3c_d[:])
        gbe_sb = {}
        for nm, w in gbe_w.items():
            t = const.tile([1, w], f32)
            nc.sync.dma_start(t[:], gbe[nm][:])
            gbe_sb[nm] = t

        # ---- one-hot stationaries: built once, SBUF-resident ----
        stat = const.tile([128, NCH, 128], stat_dt)
        for k in range(NCH):
            nc.vector.tensor_scalar(
                out=stat[:, k, :], in0=iota_sb[:], scalar1=dcol_sb[:, k:k + 1],
                scalar2=None, op0=AT.is_equal)

        # ---- propagation ----
        def prop(t_i, prows, post_cb=None):
            """prows: bf16 [128, SI, 64] destination rows (node-major).
            post_cb(g, gs): called after each 8-block group's evictions."""
            t2 = full[t_i][:].rearrange("(a b) e -> a (b e)", b=2)  # [N/2, 128] bf16
            Rs = []
            for w in range(NCALL):
                G = gp.tile([128, GC, 128], bf16, tag="G")
                nc.gpsimd.dma_gather(G[:], t2,
                                     gidx_sb[:, w * (GCALL // 16):(w + 1) * (GCALL // 16)],
                                     GCALL, GCALL, 128, queue_num=w % NQ)
                ws = slice(w * GC, (w + 1) * GC)
                T = tp2.tile([128, GC, 64], bf16, tag="T")
                nc.vector.tensor_tensor(
                    out=T[:], in0=G[:, :, 64:128],
                    in1=wo_sb[:, ws, None].to_broadcast([128, GC, 64]), op=AT.mult)
                R = rp.tile([128, GC, 64], bf16, tag="R")
                nc.vector.tensor_tensor(
                    out=R[:], in0=G[:, :, 0:64],
                    in1=we_sb[:, ws, None].to_broadcast([128, GC, 64]), op=AT.mult)
                nc.vector.tensor_tensor(out=R[:], in0=R[:], in1=T[:], op=AT.add)
                Rs.append(R)
            for b, (k0, k1) in enumerate(blocks):
                ps = pp.tile([128, 64], f32, tag="red")
                for k in range(k0, k1):
                    nc.tensor.matmul(ps[:], lhsT=stat[:, k, :],
                                     rhs=Rs[k // GC][:, k % GC, :],
                                     start=(k == k0), stop=(k == k1 - 1))
                nc.scalar.copy(out=prows[:, b, :], in_=ps[:])
                if post_cb is not None and b % 8 == 7:
                    post_cb(b // 8, slice(b - 7, b + 1))

        # ---- BatchNorm helpers ----
        def bn_coeffs(sums, cmid, g_t, be_t, st_i):
            F = 2 * cmid
            ps = pp1.tile([1, 512], f32, tag="bnps")
            nc.tensor.matmul(ps[:, 0:2 * F], lhsT=ones_k[:], rhs=sums[:, 0:2 * F],
                             start=True, stop=True)
            stt = sb.tile([1, 512], f32, tag="bnstt")
            nc.vector.tensor_copy(out=stt[:, 0:2 * F], in_=ps[:, 0:2 * F])
            if 2 * F < 512:
                nc.vector.memset(stt[:, 2 * F:], 0.0)
            nc.sync.dma_start(st_in[st_i][:], stt[:])
            nc.gpsimd.collective_compute(
                "AllReduce", AT.add, replica_groups=RG,
                ins=[st_in[st_i][:].opt()], outs=[st_out[st_i][:].opt()])
            stf = sb.tile([1, 512], f32, tag="bnstf")
            nc.sync.dma_start(stf[:], st_out[st_i][:])
            cs = sb.tile([1, 8 * cmid], f32, tag="bncs")
            nc.vector.tensor_tensor(out=cs[:, 0:cmid], in0=stf[:, 0:cmid],
                                    in1=stf[:, cmid:F], op=AT.add)
            nc.vector.tensor_tensor(out=cs[:, cmid:2 * cmid], in0=stf[:, F:F + cmid],
                                    in1=stf[:, F + cmid:2 * F], op=AT.add)
            inv_n = 1.0 / float(B * N)
            mu = cs[:, 4 * cmid:5 * cmid]
            nc.vector.tensor_scalar_mul(mu, cs[:, 0:cmid], inv_n)
            msq = cs[:, 5 * cmid:6 * cmid]
            nc.vector.tensor_scalar_mul(msq, cs[:, cmid:2 * cmid], inv_n)
            var = cs[:, 6 * cmid:7 * cmid]
            nc.vector.tensor_tensor(out=var, in0=mu, in1=mu, op=AT.mult)
            nc.vector.tensor_tensor(out=var, in0=msq, in1=var, op=AT.subtract)
            nc.vector.tensor_scalar_add(var, var, EPS)
            std = cs[:, 7 * cmid:8 * cmid]
            nc.scalar.sqrt(std, var)
            rstd = cs[:, 6 * cmid:7 * cmid]
            nc.vector.reciprocal(rstd, std)
            s_ = cs[:, 2 * cmid:3 * cmid]
            nc.vector.tensor_tensor(out=s_, in0=g_t[:], in1=rstd, op=AT.mult)
            o_ = cs[:, 3 * cmid:4 * cmid]
            nc.vector.tensor_tensor(out=o_, in0=mu, in1=s_, op=AT.mult)
            nc.vector.tensor_tensor(out=o_, in0=be_t[:], in1=o_, op=AT.subtract)
            sf = sb.tile([1, 512], f32, tag="bnsf")
            nc.vector.tensor_copy(out=sf[:, 0:cmid], in_=s_)
            nc.vector.tensor_copy(out=sf[:, cmid:F], in_=s_)
            nc.vector.tensor_copy(out=sf[:, F:F + cmid], in_=o_)
            nc.vector.tensor_copy(out=sf[:, F + cmid:2 * F], in_=o_)
            psb = pp1.tile([128, 512], f32, tag="bnps")
            nc.tensor.matmul(psb[:, 0:2 * F], lhsT=ones_m[:], rhs=sf[:, 0:2 * F],
                             start=True, stop=True)
            rep = sb.tile([128, 512], f32, tag="bnrep")
            nc.vector.tensor_copy(out=rep[:, 0:2 * F], in_=psb[:, 0:2 * F])
            return rep

        def make_stats_cb(prows, o_tile, a_view, sums, fused4=False):
            """o = p + a per group; accumulate sum(o), sum(o^2) into sums."""
            def cb(g, gs):
                if fused4:
                    nc.vector.tensor_tensor(
                        out=o_tile[:, gs, :].rearrange("p i (b c) -> p i b c", b=2),
                        in0=prows[:, gs, :].rearrange("p i (b c) -> p i b c", b=2),
                        in1=a_view[:, gs], op=AT.add)
                else:
                    nc.vector.tensor_tensor(out=o_tile[:, gs, :], in0=prows[:, gs, :],
                                            in1=a_view[:, gs], op=AT.add)
                red1 = tp2.tile([128, 64], f32, tag="bnred2")
                nc.vector.tensor_reduce(
                    out=red1[:], in_=o_tile[:, gs, :].rearrange("p i c -> p c i"),
                    axis=mybir.AxisListType.X, op=AT.add)
                nc.vector.tensor_tensor(out=sums[:, 0:64], in0=sums[:, 0:64],
                                        in1=red1[:], op=AT.add)
                sq = tp2.tile([128, 8, 64], bf16, tag="bnsq")
                nc.vector.tensor_tensor(out=sq[:], in0=o_tile[:, gs, :],
                                        in1=o_tile[:, gs, :], op=AT.mult)
                red2 = tp2.tile([128, 64], f32, tag="bnred2")
                nc.vector.tensor_reduce(
                    out=red2[:], in_=sq[:].rearrange("p i c -> p c i"),
                    axis=mybir.AxisListType.X, op=AT.add)
                nc.vector.tensor_tensor(out=sums[:, 64:128], in0=sums[:, 64:128],
                                        in1=red2[:], op=AT.add)
            return cb

        def bn_apply(orows, sums, cmid, g_t, be_t, st_i):
            F = 2 * cmid
            rep = bn_coeffs(sums, cmid, g_t, be_t, st_i)
            zr = sb.tile([128, SI, F], bf16, tag="Z")
            nc.vector.tensor_tensor(out=zr[:], in0=orows[:],
                                    in1=rep[:, None, 0:F].to_broadcast([128, SI, F]), op=AT.mult)
            nc.vector.tensor_tensor(out=zr[:], in0=zr[:],
                                    in1=rep[:, None, F:2 * F].to_broadcast([128, SI, F]), op=AT.add)
            nc.vector.tensor_scalar_max(zr[:], zr[:], 0.0)
            return zr

        # ================= Layer 1 dense (f32 in, bf16 out) =================
        # v-pass first so AllGather0 starts early; a/u pass runs under it.
        vt = sb.tile([128, SI, 2, 32], bf16, tag="V")
        for g in range(SI // 8):
            for b in range(B):
                xtb = og.tile([128, 1024], f32, tag="xtb")
                nc.sync.dma_start(xtb[:], xT[b, :, g * 1024:(g + 1) * 1024])
                for j in range(8):
                    i = g * 8 + j
                    psd = pd.tile([128, 256], f32, tag="dps")
                    nc.tensor.matmul(psd[:, 0:32], lhsT=xtb[:, j * 128:(j + 1) * 128],
                                     rhs=w1_sb[:, 64:96], start=True, stop=True)
                    nc.scalar.copy(out=vt[:, i, b, :], in_=psd[:, 0:32])
        for b in range(B):
            nc.sync.dma_start(shard_tile_ap(0)[:, :, b * 32:(b + 1) * 32],
                              vt[:, :, b, :])
        nc.gpsimd.collective_compute("AllGather", AT.bypass, replica_groups=RG,
                                     ins=[shard[0][:].opt()], outs=[full[0][:].opt()])
        # a/u pass: hold1 [128, SI, 2, 64] = [a32 | u32] per (tile, batch)
        hold1 = sb.tile([128, SI, 2, 64], bf16, tag="hold")
        for g in range(SI // 8):
            for b in range(B):
                xtb = og.tile([128, 1024], f32, tag="xtb")
                nc.sync.dma_start(xtb[:], xT[b, :, g * 1024:(g + 1) * 1024])
                for j in range(8):
                    i = g * 8 + j
                    psd = pd.tile([128, 256], f32, tag="dps")
                    nc.tensor.matmul(psd[:, 0:64], lhsT=xtb[:, j * 128:(j + 1) * 128],
                                     rhs=w1_sb[:, 0:64], start=True, stop=True)
                    nc.scalar.copy(out=hold1[:, i, b, :], in_=psd[:, 0:64])
        p11 = sb.tile([128, SI, 64], bf16, tag="P")
        prop(0, p11)
        q1 = sb.tile([128, SI, 64], bf16, tag="Q")
        nc.vector.scalar_tensor_tensor(
            out=q1[:].rearrange("p i (b c) -> p i b c", b=2),
            in0=p11[:].rearrange("p i (b c) -> p i b c", b=2),
            scalar=2.0, in1=hold1[:, :, :, 32:64], op0=AT.mult, op1=AT.add)
        nc.sync.dma_start(shard_tile_ap(1), q1[:])
        nc.gpsimd.collective_compute("AllGather", AT.bypass, replica_groups=RG,
                                     ins=[shard[1][:].opt()], outs=[full[1][:].opt()])
        sums1 = sb.tile([128, 512], f32, tag="bnsums")
        nc.vector.memset(sums1[:, 0:128], 0.0)
        p12 = sb.tile([128, SI, 64], bf16, tag="P")
        o1 = sb.tile([128, SI, 64], bf16, tag="O")
        prop(1, p12, post_cb=make_stats_cb(p12, o1, hold1[:, :, :, 0:32], sums1,
                                           fused4=True))
        z2 = bn_apply(o1, sums1, C_MID, gbe_sb["g1"], gbe_sb["be1"], 0)

        # ================= Layer 2 (bf16) =================
        # v-pass
        vt2 = sb.tile([128, SI, 64], bf16, tag="V")
        for i in range(SI):
            tp = pd.tile([64, 128], bf16, tag="tps")
            nc.tensor.transpose(out=tp[:], in_=z2[:, i, :], identity=ident[:])
            ztc = tp2.tile([64, 128], bf16, tag="ztc")
            nc.scalar.copy(out=ztc[:], in_=tp[:])
            psd = pd.tile([128, 256], f32, tag="dps")
            nc.tensor.matmul(psd[:, 0:64], lhsT=ztc[:], rhs=w2_sb[:, 128:192],
                             start=True, stop=True)
            nc.scalar.copy(out=vt2[:, i, :], in_=psd[:, 0:64])
        nc.sync.dma_start(shard_tile_ap(2), vt2[:])
        nc.gpsimd.collective_compute("AllGather", AT.bypass, replica_groups=RG,
                                     ins=[shard[2][:].opt()], outs=[full[2][:].opt()])
        # a/u pass
        hold2 = sb.tile([128, SI, 128], bf16, tag="hold")
        for i in range(SI):
            tp = pd.tile([64, 128], bf16, tag="tps")
            nc.tensor.transpose(out=tp[:], in_=z2[:, i, :], identity=ident[:])
            ztc = tp2.tile([64, 128], bf16, tag="ztc")
            nc.scalar.copy(out=ztc[:], in_=tp[:])
            psd = pd.tile([128, 256], f32, tag="dps")
            nc.tensor.matmul(psd[:, 0:128], lhsT=ztc[:], rhs=w2_sb[:, 0:128],
                             start=True, stop=True)
            nc.scalar.copy(out=hold2[:, i, :], in_=psd[:, 0:128])
        p21 = sb.tile([128, SI, 64], bf16, tag="P")
        prop(2, p21)
        q2 = sb.tile([128, SI, 64], bf16, tag="Q")
        nc.vector.scalar_tensor_tensor(out=q2[:], in0=p21[:], scalar=2.0,
                                       in1=hold2[:, :, 64:128], op0=AT.mult, op1=AT.add)
        nc.sync.dma_start(shard_tile_ap(3), q2[:])
        nc.gpsimd.collective_compute("AllGather", AT.bypass, replica_groups=RG,
                                     ins=[shard[3][:].opt()], outs=[full[3][:].opt()])
        sums2 = sb.tile([128, 512], f32, tag="bnsums")
        nc.vector.memset(sums2[:, 0:128], 0.0)
        p22 = sb.tile([128, SI, 64], bf16, tag="P")
        o2 = sb.tile([128, SI, 64], bf16, tag="O")
        prop(3, p22, post_cb=make_stats_cb(p22, o2, hold2[:, :, 0:64], sums2))
        z3 = bn_apply(o2, sums2, C_MID, gbe_sb["g2"], gbe_sb["be2"], 1)

        # ================= Layer 3 =================
        nc.sync.dma_start(shard_tile_ap(4), z3[:])
        nc.gpsimd.collective_compute("AllGather", AT.bypass, replica_groups=RG,
                                     ins=[shard[4][:].opt()], outs=[full[4][:].opt()])
        t1r = sb.tile([128, SI, 64], bf16, tag="P")
        prop(4, t1r)
        nc.sync.dma_start(shard_tile_ap(5), t1r[:])
        nc.gpsimd.collective_compute("AllGather", AT.bypass, replica_groups=RG,
                                     ins=[shard[5][:].opt()], outs=[full[5][:].opt()])
        p32 = sb.tile([128, SI, 64], bf16, tag="Q")
        prop(5, p32)

        o3 = sb.tile([128, SI, 256], bf16, tag="O3")
        acc_s = sb.tile([128, 512], f32, tag="bnsums")
        nc.vector.memset(acc_s[:], 0.0)
        for g in range(SI // 8):
            for j in range(8):
                i = g * 8 + j
                psd = pd.tile([128, 256], f32, tag="dps")
                for (rows_t, w_t, st_, sp_) in ((z3, w3a, True, False),
                                                (t1r, w3b, False, False),
                                                (p32, w3c, False, True)):
                    tp = pd.tile([64, 128], bf16, tag="tps")
                    nc.tensor.transpose(out=tp[:], in_=rows_t[:, i, :], identity=ident[:])
                    ztc = tp2.tile([64, 128], bf16, tag="ztc")
                    nc.scalar.copy(out=ztc[:], in_=tp[:])
                    nc.tensor.matmul(psd[:], lhsT=ztc[:], rhs=w_t[:], start=st_, stop=sp_)
                nc.scalar.copy(out=o3[:, i, :], in_=psd[:])
            gs = slice(g * 8, (g + 1) * 8)
            red = sb.tile([128, 512], f32, tag="bnred")
            nc.vector.tensor_reduce(out=red[:, 0:256],
                                    in_=o3[:, gs, :].rearrange("p j c -> p c j"),
                                    axis=mybir.AxisListType.X, op=AT.add)
            nc.vector.memset(red[:, 256:512], 0.0)
            for h in range(2):
                sqh = og.tile([128, 4, 256], bf16, tag="o3sq")
                nc.vector.tensor_tensor(out=sqh[:], in0=o3[:, g * 8 + h * 4:g * 8 + (h + 1) * 4, :],
                                        in1=o3[:, g * 8 + h * 4:g * 8 + (h + 1) * 4, :], op=AT.mult)
                red2 = sb.tile([128, 256], f32, tag="bnred2b")
                nc.vector.tensor_reduce(out=red2[:], in_=sqh[:].rearrange("p j c -> p c j"),
                                        axis=mybir.AxisListType.X, op=AT.add)
                nc.vector.tensor_tensor(out=red[:, 256:512], in0=red[:, 256:512],
                                        in1=red2[:], op=AT.add)
            nc.vector.tensor_tensor(out=acc_s[:], in0=acc_s[:], in1=red[:], op=AT.add)
        rep3 = bn_coeffs(acc_s, C_OUT, gbe_sb["g3"], gbe_sb["be3"], 2)

        for g in range(SI // 4):
            gs = slice(g * 4, (g + 1) * 4)
            zc = og.tile([128, 4, 256], f32, tag="zc")
            nc.vector.tensor_tensor(out=zc[:], in0=o3[:, gs, :],
                                    in1=rep3[:, None, 0:256].to_broadcast([128, 4, 256]),
                                    op=AT.mult)
            nc.vector.tensor_tensor(out=zc[:], in0=zc[:],
                                    in1=rep3[:, None, 256:512].to_broadcast([128, 4, 256]),
                                    op=AT.add)
            nc.vector.tensor_scalar_max(zc[:], zc[:], 0.0)
            xc = og.tile([128, 4, 256], bf16, tag="xc")
            nc.sync.dma_start(xc[:], xrt[:, gs, :])
            nc.vector.tensor_tensor(out=zc[:], in0=zc[:], in1=xc[:], op=AT.add)
            nc.vector.tensor_scalar_max(zc[:], zc[:], 0.0)
            nc.sync.dma_start(out_d[:, gs, :], zc[:])

    nc.compile()
    return nc


def kernel(x, edge_index, edge_weight,
           W1, b1, g1, be1, W2, b2, g2, be2, W3, b3, g3, be3):
    from concourse.bass_utils import run_bass_kernel_spmd

    x = np.asarray(x, np.float32)
    in_maps, meta = _host_prep(x, edge_index, edge_weight)
    wts = _pack_weights(W1, W2, W3, g1, be1, g2, be2, g3, be3)
    for m in in_maps:
        m.update(wts)

    key = (meta["L2g"], meta["NCH"], tuple(k for _, k in meta["blocks"]))
    if key not in _CACHE:
        _CACHE[key] = _build_program(meta)
    nc = _CACHE[key]

    trace = os.environ.get("BK_TRACE", "0") == "1"
    kw = {"trace": True} if trace else {}
    res = run_bass_kernel_spmd(nc, in_maps, list(range(NC)), **kw)
    if trace:
        print(f"HW exec time: {res.exec_time_ns} ns (mean {res.mean_exec_time_ns})")

    out = np.empty((B, N, 128), np.float32)
    for c in range(NC):
        oc = res.results[c]["out"]  # [128, SI, 256] tile layout
        rows = oc.transpose(1, 0, 2).reshape(S, 256)  # node = i*128 + p
        out[0, c * S:(c + 1) * S, :] = rows[:, 0:128]
        out[1, c * S:(c + 1) * S, :] = rows[:, 128:256]
    kernel._last_results = res
    return out


# revision 14
# speedup vs baseline: 1.8475x; 1.0303x over previous
"""Trainium2 Bass kernel for nn_BottleneckBlock (Chebyshev GNN bottleneck block).

v3: bf16 tables/gathers, fp8 SBUF-resident one-hot stationaries,
v-first dense passes (AllGather starts before the a/u pass),
BN stats folded into the propagation window (per-block-group),
o3 kept in SBUF (no DRAM round-trip), deeper gather buffering.

Math restructure (per Chebyshev layer, K=3) as baseline:
    out = x (W0 - W2) + L(x W1 + 2 L (x W2))  for layers 1, 2
    layer 3 standard recursion.  Biases before BatchNorm cancel.
Sharding: nodes split 8 ways; per-prop AllGather of the bf16 table in
permuted row order (shard writes contiguous); dma_gather of paired
256B rows; TensorE one-hot reduction per 128-dst block.
"""

import os
import contextlib
import numpy as np

NC = 8
N = 49152
B = 2
C_MID = 32
C_OUT = 128
EPS = 1e-5
S = N // NC           # 6144 nodes per core
SI = S // 128         # 48 dst blocks
GCALL = int(os.environ.get("BK_GCALL", "1024"))
GC = GCALL // 128     # chunks per gather call
NQ = int(os.environ.get("BK_NQ", "4"))
STAT_FP8 = os.environ.get("BK_STATFP8", "1") == "1"

_CACHE = {}


def _wrap16(idx):
    a = np.asarray(idx, np.int16).reshape(-1, 16).T
    return np.ascontiguousarray(np.tile(a, (8, 1)))


def _perm_row(node):
    """Global node id -> permuted table row (per-core block-interleaved)."""
    c = node // S
    nl = node % S
    return c * S + (nl % 128) * SI + nl // 128


def _host_prep(x, edge_index, edge_weight):
    import ml_dtypes
    bf = ml_dtypes.bfloat16

    src = np.asarray(edge_index[0], np.int64)
    dst = np.asarray(edge_index[1], np.int64)
    ew = np.asarray(edge_weight, np.float32)

    deg = np.bincount(src, weights=ew.astype(np.float64), minlength=N).astype(np.float32)
    dinv = np.where(deg > 0, 1.0 / np.sqrt(np.maximum(deg, 1e-30)), 0.0).astype(np.float32)
    nw = (-dinv[src] * ew * dinv[dst]).astype(np.float32)

    per_core = []
    for c in range(NC):
        sel = np.nonzero((dst >= c * S) & (dst < (c + 1) * S))[0]
        d_loc = (dst[sel] - c * S).astype(np.int64)
        order = np.argsort(d_loc // 128, kind="stable")
        per_core.append((sel[order], d_loc[order]))

    kb = np.zeros(SI, np.int64)  # chunks per block (unified across cores)
    for c in range(NC):
        _, d_loc = per_core[c]
        cnt = np.bincount(d_loc // 128, minlength=SI)
        kb = np.maximum(kb, -(-cnt // 128))
    kb = np.maximum(kb, 1)
    k_end = np.cumsum(kb)
    k_off = k_end - kb
    NCH = int(k_end[-1])
    blocks = [(int(k_off[b]), int(k_end[b])) for b in range(SI)]
    L2 = NCH * 128
    L2g = -(-L2 // GCALL) * GCALL
    NCALL = L2g // GCALL
    NCHP = L2g // 128    # padded chunk count (we/wo sized to this)

    in_maps = []
    for c in range(NC):
        sel, d_loc = per_core[c]
        g16 = np.zeros(L2g, np.int16)
        nwe = np.zeros(L2g, np.float32)
        nwo = np.zeros(L2g, np.float32)
        dcol = np.full((128, NCHP), -1.0, np.float32)
        cnt = np.bincount(d_loc // 128, minlength=SI)
        eo = np.concatenate([[0], np.cumsum(cnt)])
        for b in range(SI):
            e_ids = sel[eo[b]:eo[b + 1]]
            dl = d_loc[eo[b]:eo[b + 1]]
            o = int(k_off[b]) * 128
            k = e_ids.size
            rowp = _perm_row(src[e_ids])
            g16[o:o + k] = (rowp >> 1).astype(np.int16)
            par = (rowp & 1).astype(bool)
            w = nw[e_ids]
            nwe[o:o + k] = np.where(~par, w, 0.0)
            nwo[o:o + k] = np.where(par, w, 0.0)
            colv = np.full(int(kb[b]) * 128, -1.0, np.float32)
            colv[:k] = (dl % 128).astype(np.float32)
            dcol[:, int(k_off[b]):int(k_end[b])] = colv.reshape(-1, 128).T
        sl = slice(c * S, (c + 1) * S)
        xs = np.asarray(x[:, sl, :], np.float32)          # [2, S, 128]
        xr = np.concatenate([xs[0], xs[1]], axis=1)       # [S, 256] fused rows
        xrt = np.ascontiguousarray(
            xr.reshape(SI, 128, 256).transpose(1, 0, 2))  # [128, SI, 256]
        in_maps.append({
            "gidx": _wrap16(g16),
            "we": np.ascontiguousarray(nwe.reshape(-1, 128).T.astype(bf)),
            "wo": np.ascontiguousarray(nwo.reshape(-1, 128).T.astype(bf)),
            "dstcol": np.ascontiguousarray(dcol),
            "xT": np.ascontiguousarray(xs.transpose(0, 2, 1)),   # [2, 128, S] f32
            "xrt": np.ascontiguousarray(xrt.astype(bf)),          # bf16
        })

    iota = np.ascontiguousarray(
        np.broadcast_to(np.arange(128, dtype=np.float32), (128, 128)))
    for m in in_maps:
        m["iota"] = iota

    meta = {"L2g": L2g, "NCALL": NCALL, "NCH": NCH, "NCHP": NCHP, "blocks": blocks}
    return in_maps, meta


def _pack_weights(W1, W2, W3, g1, be1, g2, be2, g3, be3):
    import ml_dtypes
    bf = ml_dtypes.bfloat16
    W1 = np.asarray(W1, np.float32)
    W2 = np.asarray(W2, np.float32)
    W3 = np.asarray(W3, np.float32)
    # layer1: [a|u|v] = [W0-W2 | W1 | W2]
    w1cat = np.concatenate([W1[0] - W1[2], W1[1], W1[2]], axis=1)  # [128, 96] f32

    def fuse(w):  # [ci, co] -> [2ci, 2co] block-diag over batch
        ci, co = w.shape
        out = np.zeros((2 * ci, 2 * co), np.float32)
        out[:ci, :co] = w
        out[ci:, co:] = w
        return out

    w2bundle = np.concatenate([fuse(W2[0] - W2[2]), fuse(W2[1]), fuse(W2[2])], axis=1)
    return {
        "w1cat": np.ascontiguousarray(w1cat),
        "w2bundle": np.ascontiguousarray(w2bundle.astype(bf)),      # [64, 192]
        "w3a": np.ascontiguousarray(fuse(W3[0] - W3[2]).astype(bf)),  # [64, 256]
        "w3b": np.ascontiguousarray(fuse(W3[1]).astype(bf)),
        "w3c": np.ascontiguousarray(fuse(2.0 * W3[2]).astype(bf)),
        "g1": np.asarray(g1, np.float32)[None, :], "be1": np.asarray(be1, np.float32)[None, :],
        "g2": np.asarray(g2, np.float32)[None, :], "be2": np.asarray(be2, np.float32)[None, :],
        "g3": np.asarray(g3, np.float32)[None, :], "be3": np.asarray(be3, np.float32)[None, :],
    }


def _build_program(meta, debug=False):
    import concourse.bacc as bacc
    import concourse.mybir as mybir
    import concourse.tile as tile
    from concourse.library_config import mlp
    from concourse.masks import make_identity

    f32 = mybir.dt.float32
    bf16 = mybir.dt.bfloat16
    fp8 = mybir.dt.float8e4
    i16 = mybir.dt.int16
    AT = mybir.AluOpType
    stat_dt = fp8 if STAT_FP8 else bf16
    L2g, NCALL, NCH, NCHP, blocks = (
        meta["L2g"], meta["NCALL"], meta["NCH"], meta["NCHP"], meta["blocks"])

    nc = bacc.Bacc("TRN2", target_bir_lowering=False, debug=False, num_devices=NC,
                   num_swdge_queues=NQ,
                   dynamic_dma_scratch_size=int(os.environ.get("BK_SCRATCH", "16384")))

    # ---- I/O ----
    gidx = nc.dram_tensor("gidx", [128, L2g // 16], i16, kind="ExternalInput")
    we_d = nc.dram_tensor("we", [128, NCHP], bf16, kind="ExternalInput")
    wo_d = nc.dram_tensor("wo", [128, NCHP], bf16, kind="ExternalInput")
    dstcol_d = nc.dram_tensor("dstcol", [128, NCHP], f32, kind="ExternalInput")
    iota_d = nc.dram_tensor("iota", [128, 128], f32, kind="ExternalInput")
    xT = nc.dram_tensor("xT", [B, 128, S], f32, kind="ExternalInput")
    xrt = nc.dram_tensor("xrt", [128, SI, 256], bf16, kind="ExternalInput")
    w1cat = nc.dram_tensor("w1cat", [128, 96], f32, kind="ExternalInput")
    w2bundle = nc.dram_tensor("w2bundle", [64, 192], bf16, kind="ExternalInput")
    w3a_d = nc.dram_tensor("w3a", [64, 256], bf16, kind="ExternalInput")
    w3b_d = nc.dram_tensor("w3b", [64, 256], bf16, kind="ExternalInput")
    w3c_d = nc.dram_tensor("w3c", [64, 256], bf16, kind="ExternalInput")
    gbe_w = {"g1": 32, "be1": 32, "g2": 32, "be2": 32, "g3": 128, "be3": 128}
    gbe = {nm: nc.dram_tensor(nm, [1, w], f32, kind="ExternalInput") for nm, w in gbe_w.items()}
    out_d = nc.dram_tensor("out", [128, SI, 256], f32, kind="ExternalOutput")

    # ---- internal DRAM ----
    full = [nc.dram_tensor(f"full{i}", [N, 64], bf16, addr_space="Shared") for i in range(6)]
    shard = [nc.dram_tensor(f"shard{i}", [S, 64], bf16) for i in range(6)]
    st_in = [nc.dram_tensor(f"stin{i}", [1, 512], f32) for i in range(3)]
    st_out = [nc.dram_tensor(f"stout{i}", [1, 512], f32, addr_space="Shared") for i in range(3)]

    RG = [list(range(NC))]

    def shard_tile_ap(i):
        return shard[i][:].rearrange("(p i) e -> p i e", p=128)

    with tile.TileContext(nc) as tc, contextlib.ExitStack() as ctx:
        const = ctx.enter_context(tc.tile_pool(name="const", bufs=1))
        sb = ctx.enter_context(tc.tile_pool(name="sb", bufs=1))
        gp = ctx.enter_context(tc.tile_pool(name="gp", bufs=4))
        rp = ctx.enter_context(tc.tile_pool(name="rp", bufs=4))
        tp2 = ctx.enter_context(tc.tile_pool(name="tp2", bufs=2))
        og = ctx.enter_context(tc.tile_pool(name="og", bufs=2))
        pp = ctx.enter_context(tc.tile_pool(name="pp", bufs=3, space="PSUM"))
        pd = ctx.enter_context(tc.tile_pool(name="pd", bufs=2, space="PSUM"))
        pp1 = ctx.enter_context(tc.tile_pool(name="pp1", bufs=1, space="PSUM"))

        nc.gpsimd.load_library(mlp)

        ident = const.tile([128, 128], bf16)
        make_identity(nc, ident[:])
        ones_k = const.tile([128, 1], f32)
        nc.vector.memset(ones_k[:], 1.0)
        ones_m = const.tile([1, 128], f32)
        nc.vector.memset(ones_m[:], 1.0)
        ones_1 = const.tile([1, 1], f32)
        nc.vector.memset(ones_1[:], 1.0)

        gidx_sb = const.tile([128, L2g // 16], i16)
        we_sb = const.tile([128, NCHP], bf16)
        wo_sb = const.tile([128, NCHP], bf16)
        dcol_sb = const.tile([128, NCHP], f32)
        iota_sb = const.tile([128, 128], f32)
        nc.sync.dma_start(gidx_sb[:], gidx[:])
        nc.sync.dma_start(we_sb[:], we_d[:])
        nc.sync.dma_start(wo_sb[:], wo_d[:])
        nc.sync.dma_start(dcol_sb[:], dstcol_d[:])
        nc.sync.dma_start(iota_sb[:], iota_d[:])

        w1_sb = const.tile([128, 96], f32)
        w2_sb = const.tile([64, 192], bf16)
        w3a = const.tile([64, 256], bf16)
        w3b = const.tile([64, 256], bf16)
        w3c = const.tile([64, 256], bf16)
        nc.sync.dma_start(w1_sb[:], w1cat[:])
        nc.sync.dma_start(w2_sb[:], w2bundle[:])
        nc.sync.dma_start(w3a[:], w3a_d[:])
        nc.sync.dma_start(w3b[:], w3b_d[:])
        nc.sync.dma_start(w3c[:], w3c_d[:])
        gbe_sb = {}
        for nm, w in gbe_w.items():
            t = const.tile([1, w], f32)
            nc.sync.dma_start(t[:], gbe[nm][:])
            gbe_sb[nm] = t

        # ---- one-hot stationaries: built once, SBUF-resident ----
        stat = const.tile([128, NCH, 128], stat_dt)
        for k in range(NCH):
            nc.vector.tensor_scalar(
                out=stat[:, k, :], in0=iota_sb[:], scalar1=dcol_sb[:, k:k + 1],
                scalar2=None, op0=AT.is_equal)

        # ---- propagation ----
        def prop(t_i, prows, post_cb=None):
            """prows: bf16 [128, SI, 64] destination rows (node-major).
            Blocks are processed as soon as their chunks' gathers complete so
            downstream DVE/PE work interleaves with the gather stream.
            post_cb(g, gs): called after each 8-block group's evictions."""
            t2 = full[t_i][:].rearrange("(a b) e -> a (b e)", b=2)  # [N/2, 128] bf16
            Rs = []
            done_b = 0

            def run_block(b):
                k0, k1 = blocks[b]
                ps = pp.tile([128, 64], f32, tag="red")
                for k in range(k0, k1):
                    nc.tensor.matmul(ps[:], lhsT=stat[:, k, :],
                                     rhs=Rs[k // GC][:, k % GC, :],
                                     start=(k == k0), stop=(k == k1 - 1))
                nc.scalar.copy(out=prows[:, b, :], in_=ps[:])
                if post_cb is not None and b % 8 == 7:
                    post_cb(b // 8, slice(b - 7, b + 1))

            for w in range(NCALL):
                G = gp.tile([128, GC, 128], bf16, tag="G")
                nc.gpsimd.dma_gather(G[:], t2,
                                     gidx_sb[:, w * (GCALL // 16):(w + 1) * (GCALL // 16)],
                                     GCALL, GCALL, 128, queue_num=w % NQ)
                ws = slice(w * GC, (w + 1) * GC)
                T = tp2.tile([128, GC, 64], bf16, tag="T")
                nc.vector.tensor_tensor(
                    out=T[:], in0=G[:, :, 64:128],
                    in1=wo_sb[:, ws, None].to_broadcast([128, GC, 64]), op=AT.mult)
                R = rp.tile([128, GC, 64], bf16, tag="R")
                nc.vector.tensor_tensor(
                    out=R[:], in0=G[:, :, 0:64],
                    in1=we_sb[:, ws, None].to_broadcast([128, GC, 64]), op=AT.mult)
                nc.vector.tensor_tensor(out=R[:], in0=R[:], in1=T[:], op=AT.add)
                Rs.append(R)
                hi = (w + 1) * GC
                while done_b < SI and blocks[done_b][1] <= hi:
                    run_block(done_b)
                    done_b += 1
            while done_b < SI:
                run_block(done_b)
                done_b += 1

        # ---- BatchNorm helpers ----
        def bn_sf(sums, cmid, g_t, be_t, st_i):
            F = 2 * cmid
            ps = pp1.tile([1, 512], f32, tag="bnps")
            nc.tensor.matmul(ps[:, 0:2 * F], lhsT=ones_k[:], rhs=sums[:, 0:2 * F],
                             start=True, stop=True)
            stt = sb.tile([1, 512], f32, tag="bnstt")
            nc.vector.tensor_copy(out=stt[:, 0:2 * F], in_=ps[:, 0:2 * F])
            if 2 * F < 512:
                nc.vector.memset(stt[:, 2 * F:], 0.0)
            nc.sync.dma_start(st_in[st_i][:], stt[:])
            nc.gpsimd.collective_compute(
                "AllReduce", AT.add, replica_groups=RG,
                ins=[st_in[st_i][:].opt()], outs=[st_out[st_i][:].opt()])
            stf = sb.tile([1, 512], f32, tag="bnstf")
            nc.sync.dma_start(stf[:], st_out[st_i][:])
            cs = sb.tile([1, 8 * cmid], f32, tag="bncs")
            nc.vector.tensor_tensor(out=cs[:, 0:cmid], in0=stf[:, 0:cmid],
                                    in1=stf[:, cmid:F], op=AT.add)
            nc.vector.tensor_tensor(out=cs[:, cmid:2 * cmid], in0=stf[:, F:F + cmid],
                                    in1=stf[:, F + cmid:2 * F], op=AT.add)
            inv_n = 1.0 / float(B * N)
            mu = cs[:, 4 * cmid:5 * cmid]
            nc.vector.tensor_scalar_mul(mu, cs[:, 0:cmid], inv_n)
            msq = cs[:, 5 * cmid:6 * cmid]
            nc.vector.tensor_scalar_mul(msq, cs[:, cmid:2 * cmid], inv_n)
            var = cs[:, 6 * cmid:7 * cmid]
            nc.vector.tensor_tensor(out=var, in0=mu, in1=mu, op=AT.mult)
            nc.vector.tensor_tensor(out=var, in0=msq, in1=var, op=AT.subtract)
            nc.vector.tensor_scalar_add(var, var, EPS)
            std = cs[:, 7 * cmid:8 * cmid]
            nc.scalar.sqrt(std, var)
            rstd = cs[:, 6 * cmid:7 * cmid]
            nc.vector.reciprocal(rstd, std)
            s_ = cs[:, 2 * cmid:3 * cmid]
            nc.vector.tensor_tensor(out=s_, in0=g_t[:], in1=rstd, op=AT.mult)
            o_ = cs[:, 3 * cmid:4 * cmid]
            nc.vector.tensor_tensor(out=o_, in0=mu, in1=s_, op=AT.mult)
            nc.vector.tensor_tensor(out=o_, in0=be_t[:], in1=o_, op=AT.subtract)
            sf = sb.tile([1, 512], f32, tag="bnsf")
            nc.vector.tensor_copy(out=sf[:, 0:cmid], in_=s_)
            nc.vector.tensor_copy(out=sf[:, cmid:F], in_=s_)
            nc.vector.tensor_copy(out=sf[:, F:F + cmid], in_=o_)
            nc.vector.tensor_copy(out=sf[:, F + cmid:2 * F], in_=o_)
            return sf

        def bn_rep(sf, F):
            psb = pp1.tile([128, 512], f32, tag="bnps")
            nc.tensor.matmul(psb[:, 0:2 * F], lhsT=ones_m[:], rhs=sf[:, 0:2 * F],
                             start=True, stop=True)
            rep = sb.tile([128, 512], f32, tag="bnrep")
            nc.vector.tensor_copy(out=rep[:, 0:2 * F], in_=psb[:, 0:2 * F])
            return rep

        def bn_cols(sf, F):
            # per-partition BN coeff columns for channel-major apply
            psc = pp1.tile([F, 2], f32, tag="bnps")
            nc.tensor.matmul(psc[:, 0:1], lhsT=sf[:, 0:F], rhs=ones_1[:],
                             start=True, stop=True)
            nc.tensor.matmul(psc[:, 1:2], lhsT=sf[:, F:2 * F], rhs=ones_1[:],
                             start=True, stop=True)
            cols = sb.tile([F, 2], f32, tag="bncols")
            nc.scalar.copy(out=cols[:], in_=psc[:])
            return cols

        def make_stats_cb(prows, a_view, sums, fused4=False, t_dst=None):
            """o = p + a in place on prows; accumulate stats; optionally
            transpose the o tiles into t_dst ([64|128]-partition slices)."""
            def cb(g, gs):
                if fused4:
                    nc.vector.tensor_tensor(
                        out=prows[:, gs, :].rearrange("p i (b c) -> p i b c", b=2),
                        in0=prows[:, gs, :].rearrange("p i (b c) -> p i b c", b=2),
                        in1=a_view[:, gs], op=AT.add)
                else:
                    nc.vector.tensor_tensor(out=prows[:, gs, :], in0=prows[:, gs, :],
                                            in1=a_view[:, gs], op=AT.add)
                red1 = tp2.tile([128, 64], f32, tag="bnred2")
                nc.vector.tensor_reduce(
                    out=red1[:], in_=prows[:, gs, :].rearrange("p i c -> p c i"),
                    axis=mybir.AxisListType.X, op=AT.add)
                nc.vector.tensor_tensor(out=sums[:, 0:64], in0=sums[:, 0:64],
                                        in1=red1[:], op=AT.add)
                sq = tp2.tile([128, 8, 64], bf16, tag="bnsq")
                nc.vector.tensor_tensor(out=sq[:], in0=prows[:, gs, :],
                                        in1=prows[:, gs, :], op=AT.mult)
                red2 = tp2.tile([128, 64], f32, tag="bnred2")
                nc.vector.tensor_reduce(
                    out=red2[:], in_=sq[:].rearrange("p i c -> p c i"),
                    axis=mybir.AxisListType.X, op=AT.add)
                nc.vector.tensor_tensor(out=sums[:, 64:128], in0=sums[:, 64:128],
                                        in1=red2[:], op=AT.add)
                if t_dst is not None:
                    for i in range(gs.start, gs.stop):
                        tp = pd.tile([64, 128], bf16, tag="tps")
                        nc.tensor.transpose(out=tp[:], in_=prows[:, i, :],
                                            identity=ident[:])
                        nc.scalar.copy(out=t_dst[:, i, :], in_=tp[:])
            return cb

        def transpose_cb(srcs_dsts):
            """cb transposing [128,64] tiles of (src, dst-slice) pairs."""
            def cb(g, gs):
                for src_t, dst_t in srcs_dsts:
                    for i in range(gs.start, gs.stop):
                        tp = pd.tile([64, 128], bf16, tag="tps")
                        nc.tensor.transpose(out=tp[:], in_=src_t[:, i, :],
                                            identity=ident[:])
                        nc.scalar.copy(out=dst_t[:, i, :], in_=tp[:])
            return cb

        def bn_apply(orows, sums, cmid, g_t, be_t, st_i):
            F = 2 * cmid
            rep = bn_rep(bn_sf(sums, cmid, g_t, be_t, st_i), F)
            zr = sb.tile([128, SI, F], bf16, tag="Z")
            nc.vector.tensor_tensor(out=zr[:], in0=orows[:],
                                    in1=rep[:, None, 0:F].to_broadcast([128, SI, F]), op=AT.mult)
            nc.vector.tensor_tensor(out=zr[:], in0=zr[:],
                                    in1=rep[:, None, F:2 * F].to_broadcast([128, SI, F]), op=AT.add)
            nc.vector.tensor_scalar_max(zr[:], zr[:], 0.0)
            return zr

        # ================= Layer 1 dense (f32 in, bf16 out) =================
        # v-pass first so AllGather0 starts early; a/u pass runs under it.
        vt = sb.tile([128, SI, 2, 32], bf16, tag="V")
        for g in range(SI // 4):
            for b in range(B):
                xtb = og.tile([128, 512], f32, tag="xtb")
                nc.sync.dma_start(xtb[:], xT[b, :, g * 512:(g + 1) * 512])
                for j in range(4):
                    i = g * 4 + j
                    psd = pd.tile([128, 256], f32, tag="dps")
                    nc.tensor.matmul(psd[:, 0:32], lhsT=xtb[:, j * 128:(j + 1) * 128],
                                     rhs=w1_sb[:, 64:96], start=True, stop=True)
                    nc.scalar.copy(out=vt[:, i, b, :], in_=psd[:, 0:32])
        for b in range(B):
            nc.sync.dma_start(shard_tile_ap(0)[:, :, b * 32:(b + 1) * 32],
                              vt[:, :, b, :])
        nc.gpsimd.collective_compute("AllGather", AT.bypass, replica_groups=RG,
                                     ins=[shard[0][:].opt()], outs=[full[0][:].opt()])
        # a/u pass: hold1 [128, SI, 2, 64] = [a32 | u32] per (tile, batch)
        hold1 = sb.tile([128, SI, 2, 64], bf16, tag="hold")
        for g in range(SI // 4):
            for b in range(B):
                xtb = og.tile([128, 512], f32, tag="xtb")
                nc.sync.dma_start(xtb[:], xT[b, :, g * 512:(g + 1) * 512])
                for j in range(4):
                    i = g * 4 + j
                    psd = pd.tile([128, 256], f32, tag="dps")
                    nc.tensor.matmul(psd[:, 0:64], lhsT=xtb[:, j * 128:(j + 1) * 128],
                                     rhs=w1_sb[:, 0:64], start=True, stop=True)
                    nc.scalar.copy(out=hold1[:, i, b, :], in_=psd[:, 0:64])
        p11 = sb.tile([128, SI, 64], bf16, tag="P")
        prop(0, p11)
        # q1 in place on p11
        nc.vector.scalar_tensor_tensor(
            out=p11[:].rearrange("p i (b c) -> p i b c", b=2),
            in0=p11[:].rearrange("p i (b c) -> p i b c", b=2),
            scalar=2.0, in1=hold1[:, :, :, 32:64], op0=AT.mult, op1=AT.add)
        nc.sync.dma_start(shard_tile_ap(1), p11[:])
        nc.gpsimd.collective_compute("AllGather", AT.bypass, replica_groups=RG,
                                     ins=[shard[1][:].opt()], outs=[full[1][:].opt()])
        sums1 = sb.tile([128, 512], f32, tag="bnsums")
        nc.vector.memset(sums1[:, 0:128], 0.0)
        # channel-major transposed tiles (partitions 0:64):
        # TTa: o1T/z2T then z3T; TTb: t1rT.
        TTa = sb.tile([64, SI, 128], bf16, tag="TTa")
        TTb = sb.tile([64, SI, 128], bf16, tag="TTb")
        p12 = sb.tile([128, SI, 64], bf16, tag="P")
        prop(1, p12, post_cb=make_stats_cb(p12, hold1[:, :, :, 0:32], sums1,
                                           fused4=True, t_dst=TTa))
        sf1 = bn_sf(sums1, C_MID, gbe_sb["g1"], gbe_sb["be1"], 0)
        cols1 = bn_cols(sf1, 2 * C_MID)
        # z2T = relu(s*o1T + t) in place, channel-major, one batched op
        nc.scalar.activation(out=TTa[:], in_=TTa[:],
                             func=mybir.ActivationFunctionType.Relu,
                             bias=cols1[:, 1:2], scale=cols1[:, 0:1])

        # ================= Layer 2 (bf16, channel-major dense) =================
        # v-pass
        vt2 = sb.tile([128, SI, 64], bf16, tag="V")
        for i in range(SI):
            psd = pd.tile([128, 256], f32, tag="dps")
            nc.tensor.matmul(psd[:, 0:64], lhsT=TTa[:, i, :], rhs=w2_sb[:, 128:192],
                             start=True, stop=True)
            nc.scalar.copy(out=vt2[:, i, :], in_=psd[:, 0:64])
        nc.sync.dma_start(shard_tile_ap(2), vt2[:])
        nc.gpsimd.collective_compute("AllGather", AT.bypass, replica_groups=RG,
                                     ins=[shard[2][:].opt()], outs=[full[2][:].opt()])
        # a/u pass
        hold2 = sb.tile([128, SI, 128], bf16, tag="hold")
        for i in range(SI):
            psd = pd.tile([128, 256], f32, tag="dps")
            nc.tensor.matmul(psd[:, 0:128], lhsT=TTa[:, i, :], rhs=w2_sb[:, 0:128],
                             start=True, stop=True)
            nc.scalar.copy(out=hold2[:, i, :], in_=psd[:, 0:128])
        p21 = sb.tile([128, SI, 64], bf16, tag="P")
        prop(2, p21)
        nc.vector.scalar_tensor_tensor(out=p21[:], in0=p21[:], scalar=2.0,
                                       in1=hold2[:, :, 64:128], op0=AT.mult, op1=AT.add)
        nc.sync.dma_start(shard_tile_ap(3), p21[:])
        nc.gpsimd.collective_compute("AllGather", AT.bypass, replica_groups=RG,
                                     ins=[shard[3][:].opt()], outs=[full[3][:].opt()])
        sums2 = sb.tile([128, 512], f32, tag="bnsums")
        nc.vector.memset(sums2[:, 0:128], 0.0)
        p22 = sb.tile([128, SI, 64], bf16, tag="P")
        prop(3, p22, post_cb=make_stats_cb(p22, hold2[:, :, 0:64], sums2))
        z3 = bn_apply(p22, sums2, C_MID, gbe_sb["g2"], gbe_sb["be2"], 1)

        # ================= Layer 3 =================
        nc.sync.dma_start(shard_tile_ap(4), z3[:])
        nc.gpsimd.collective_compute("AllGather", AT.bypass, replica_groups=RG,
                                     ins=[shard[4][:].opt()], outs=[full[4][:].opt()])
        t1r = sb.tile([128, SI, 64], bf16, tag="P")
        prop(4, t1r, post_cb=transpose_cb([(z3, TTa), (t1r, TTb)]))
        nc.sync.dma_start(shard_tile_ap(5), t1r[:])
        nc.gpsimd.collective_compute("AllGather", AT.bypass, replica_groups=RG,
                                     ins=[shard[5][:].opt()], outs=[full[5][:].opt()])
        p32 = sb.tile([128, SI, 64], bf16, tag="Q")
        o3 = sb.tile([128, SI, 256], bf16, tag="O3")
        acc_s = sb.tile([128, 512], f32, tag="bnsums")
        nc.vector.memset(acc_s[:], 0.0)

        def l3_cb(g, gs):
            for i in range(gs.start, gs.stop):
                psd = pd.tile([128, 256], f32, tag="dps")
                nc.tensor.matmul(psd[:], lhsT=TTa[:, i, :], rhs=w3a[:],
                                 start=True, stop=False)
                nc.tensor.matmul(psd[:], lhsT=TTb[:, i, :], rhs=w3b[:],
                                 start=False, stop=False)
                tp = pd.tile([64, 128], bf16, tag="tps")
                nc.tensor.transpose(out=tp[:], in_=p32[:, i, :], identity=ident[:])
                ztc = tp2.tile([64, 128], bf16, tag="ztc")
                nc.scalar.copy(out=ztc[:], in_=tp[:])
                nc.tensor.matmul(psd[:], lhsT=ztc[:], rhs=w3c[:], start=False, stop=True)
                nc.scalar.copy(out=o3[:, i, :], in_=psd[:])
            red = sb.tile([128, 512], f32, tag="bnred")
            nc.vector.tensor_reduce(out=red[:, 0:256],
                                    in_=o3[:, gs, :].rearrange("p j c -> p c j"),
                                    axis=mybir.AxisListType.X, op=AT.add)
            nc.vector.memset(red[:, 256:512], 0.0)
            for h in range(2):
                sqh = og.tile([128, 4, 256], bf16, tag="o3sq")
                nc.vector.tensor_tensor(out=sqh[:], in0=o3[:, gs.start + h * 4:gs.start + (h + 1) * 4, :],
                                        in1=o3[:, gs.start + h * 4:gs.start + (h + 1) * 4, :], op=AT.mult)
                red2 = sb.tile([128, 256], f32, tag="bnred2b")
                nc.vector.tensor_reduce(out=red2[:], in_=sqh[:].rearrange("p j c -> p c j"),
                                        axis=mybir.AxisListType.X, op=AT.add)
                nc.vector.tensor_tensor(out=red[:, 256:512], in0=red[:, 256:512],
                                        in1=red2[:], op=AT.add)
            nc.vector.tensor_tensor(out=acc_s[:], in0=acc_s[:], in1=red[:], op=AT.add)

        prop(5, p32, post_cb=l3_cb)
        rep3 = bn_rep(bn_sf(acc_s, C_OUT, gbe_sb["g3"], gbe_sb["be3"], 2), 2 * C_OUT)

        for g in range(SI // 4):
            gs = slice(g * 4, (g + 1) * 4)
            zc = og.tile([128, 4, 256], f32, tag="zc")
            nc.vector.tensor_tensor(out=zc[:], in0=o3[:, gs, :],
                                    in1=rep3[:, None, 0:256].to_broadcast([128, 4, 256]),
                                    op=AT.mult)
            nc.vector.tensor_tensor(out=zc[:], in0=zc[:],
                                    in1=rep3[:, None, 256:512].to_broadcast([128, 4, 256]),
                                    op=AT.add)
            nc.scalar.activation(out=zc[:], in_=zc[:],
                                 func=mybir.ActivationFunctionType.Relu)
            xc = og.tile([128, 4, 256], bf16, tag="xc")
            nc.sync.dma_start(xc[:], xrt[:, gs, :])
            nc.vector.tensor_tensor(out=zc[:], in0=zc[:], in1=xc[:], op=AT.add)
            nc.scalar.activation(out=zc[:], in_=zc[:],
                                 func=mybir.ActivationFunctionType.Relu)
            nc.sync.dma_start(out_d[:, gs, :], zc[:])

    nc.compile()
    return nc


def kernel(x, edge_index, edge_weight,
           W1, b1, g1, be1, W2, b2, g2, be2, W3, b3, g3, be3):
    from concourse.bass_utils import run_bass_kernel_spmd

    x = np.asarray(x, np.float32)
    in_maps, meta = _host_prep(x, edge_index, edge_weight)
    wts = _pack_weights(W1, W2, W3, g1, be1, g2, be2, g3, be3)
    for m in in_maps:
        m.update(wts)

    key = (meta["L2g"], meta["NCH"], tuple(k for _, k in meta["blocks"]))
    if key not in _CACHE:
        _CACHE[key] = _build_program(meta)
    nc = _CACHE[key]

    trace = os.environ.get("BK_TRACE", "0") == "1"
    kw = {"trace": True} if trace else {}
    res = run_bass_kernel_spmd(nc, in_maps, list(range(NC)), **kw)
    if trace:
        print(f"HW exec time: {res.exec_time_ns} ns (mean {res.mean_exec_time_ns})")

    out = np.empty((B, N, 128), np.float32)
    for c in range(NC):
        oc = res.results[c]["out"]  # [128, SI, 256] tile layout
        rows = oc.transpose(1, 0, 2).reshape(S, 256)  # node = i*128 + p
        out[0, c * S:(c + 1) * S, :] = rows[:, 0:128]
        out[1, c * S:(c + 1) * S, :] = rows[:, 128:256]
    kernel._last_results = res
    return out


# revision 16
# speedup vs baseline: 1.9176x; 1.0379x over previous
"""Trainium2 Bass kernel for nn_BottleneckBlock (Chebyshev GNN bottleneck block).

v3: bf16 tables/gathers, fp8 SBUF-resident one-hot stationaries,
v-first dense passes (AllGather starts before the a/u pass),
BN stats folded into the propagation window (per-block-group),
o3 kept in SBUF (no DRAM round-trip), deeper gather buffering.

Math restructure (per Chebyshev layer, K=3) as baseline:
    out = x (W0 - W2) + L(x W1 + 2 L (x W2))  for layers 1, 2
    layer 3 standard recursion.  Biases before BatchNorm cancel.
Sharding: nodes split 8 ways; per-prop AllGather of the bf16 table in
permuted row order (shard writes contiguous); dma_gather of paired
256B rows; TensorE one-hot reduction per 128-dst block.
"""

import os
import contextlib
import numpy as np

NC = 8
N = 49152
B = 2
C_MID = 32
C_OUT = 128
EPS = 1e-5
S = N // NC           # 6144 nodes per core
SI = S // 128         # 48 dst blocks
GCALL = int(os.environ.get("BK_GCALL", "1024"))
GC = GCALL // 128     # chunks per gather call
NQ = int(os.environ.get("BK_NQ", "4"))
STAT_FP8 = os.environ.get("BK_STATFP8", "1") == "1"

_CACHE = {}


def _wrap16(idx):
    a = np.asarray(idx, np.int16).reshape(-1, 16).T
    return np.ascontiguousarray(np.tile(a, (8, 1)))


def _assign_blocks(dst):
    """Assign the 384 global 128-node blocks to (core, position) so the
    per-position max edge count across cores is minimized (cuts chunk
    padding): sort blocks by count desc, rank r -> core r%8, position r//8.
    Returns assign[c][i] = global block, and per-node lookup tables."""
    cnt = np.bincount(dst // 128, minlength=N // 128)
    order = np.argsort(-cnt, kind="stable")
    assign = np.empty((NC, SI), np.int64)
    blk_core = np.empty(N // 128, np.int64)
    blk_pos = np.empty(N // 128, np.int64)
    for r, g in enumerate(order):
        c, i = r % NC, r // NC
        assign[c][i] = g
        blk_core[g] = c
        blk_pos[g] = i
    return assign, blk_core, blk_pos


def _host_prep(x, edge_index, edge_weight):
    import ml_dtypes
    bf = ml_dtypes.bfloat16

    src = np.asarray(edge_index[0], np.int64)
    dst = np.asarray(edge_index[1], np.int64)
    ew = np.asarray(edge_weight, np.float32)

    deg = np.bincount(src, weights=ew.astype(np.float64), minlength=N).astype(np.float32)
    dinv = np.where(deg > 0, 1.0 / np.sqrt(np.maximum(deg, 1e-30)), 0.0).astype(np.float32)
    nw = (-dinv[src] * ew * dinv[dst]).astype(np.float32)

    assign, blk_core, blk_pos = _assign_blocks(dst)
    # permuted table row for a node: core*S + (node%128)*SI + position
    prow = blk_core[np.arange(N) // 128] * S + (np.arange(N) % 128) * SI \
        + blk_pos[np.arange(N) // 128]

    per_core = []
    for c in range(NC):
        sel = np.nonzero(blk_core[dst // 128] == c)[0]
        # local block position + dst col within block
        d_loc = blk_pos[dst[sel] // 128] * 128 + (dst[sel] % 128)
        order = np.argsort(d_loc // 128, kind="stable")
        per_core.append((sel[order], d_loc[order]))

    kb = np.zeros(SI, np.int64)  # chunks per block (unified across cores)
    for c in range(NC):
        _, d_loc = per_core[c]
        cnt = np.bincount(d_loc // 128, minlength=SI)
        kb = np.maximum(kb, -(-cnt // 128))
    kb = np.maximum(kb, 1)
    k_end = np.cumsum(kb)
    k_off = k_end - kb
    NCH = int(k_end[-1])
    blocks = [(int(k_off[b]), int(k_end[b])) for b in range(SI)]
    L2 = NCH * 128
    L2g = -(-L2 // GCALL) * GCALL
    NCALL = L2g // GCALL
    NCHP = L2g // 128    # padded chunk count (we/wo sized to this)

    in_maps = []
    for c in range(NC):
        sel, d_loc = per_core[c]
        g16 = np.zeros(L2g, np.int16)
        nwe = np.zeros(L2g, np.float32)
        nwo = np.zeros(L2g, np.float32)
        dcol = np.full((128, NCHP), -1.0, np.float32)
        cnt = np.bincount(d_loc // 128, minlength=SI)
        eo = np.concatenate([[0], np.cumsum(cnt)])
        for b in range(SI):
            e_ids = sel[eo[b]:eo[b + 1]]
            dl = d_loc[eo[b]:eo[b + 1]]
            o = int(k_off[b]) * 128
            k = e_ids.size
            rowp = prow[src[e_ids]]
            g16[o:o + k] = (rowp >> 1).astype(np.int16)
            par = (rowp & 1).astype(bool)
            w = nw[e_ids]
            nwe[o:o + k] = np.where(~par, w, 0.0)
            nwo[o:o + k] = np.where(par, w, 0.0)
            colv = np.full(int(kb[b]) * 128, -1.0, np.float32)
            colv[:k] = (dl % 128).astype(np.float32)
            dcol[:, int(k_off[b]):int(k_end[b])] = colv.reshape(-1, 128).T
        node_ids = (assign[c][:, None] * 128 +
                    np.arange(128)[None, :]).reshape(-1)   # position-major
        xs = np.asarray(x[:, node_ids, :], np.float32)     # [2, S, 128]
        xr = np.concatenate([xs[0], xs[1]], axis=1)       # [S, 256] fused rows
        xrt = np.ascontiguousarray(
            xr.reshape(SI, 128, 256).transpose(1, 0, 2))  # [128, SI, 256]
        in_maps.append({
            "gidx": _wrap16(g16),
            "we": np.ascontiguousarray(nwe.reshape(-1, 128).T.astype(bf)),
            "wo": np.ascontiguousarray(nwo.reshape(-1, 128).T.astype(bf)),
            "dstcol": np.ascontiguousarray(dcol),
            "xT": np.ascontiguousarray(xs.transpose(0, 2, 1)),   # [2, 128, S] f32
            "xrt": np.ascontiguousarray(xrt.astype(bf)),          # bf16
        })

    iota = np.ascontiguousarray(
        np.broadcast_to(np.arange(128, dtype=np.float32), (128, 128)))
    for m in in_maps:
        m["iota"] = iota

    meta = {"L2g": L2g, "NCALL": NCALL, "NCH": NCH, "NCHP": NCHP, "blocks": blocks,
            "assign": assign}
    return in_maps, meta


def _pack_weights(W1, W2, W3, g1, be1, g2, be2, g3, be3):
    import ml_dtypes
    bf = ml_dtypes.bfloat16
    W1 = np.asarray(W1, np.float32)
    W2 = np.asarray(W2, np.float32)
    W3 = np.asarray(W3, np.float32)
    # layer1: [a|u|v] = [W0-W2 | W1 | W2]
    w1cat = np.concatenate([W1[0] - W1[2], W1[1], W1[2]], axis=1)  # [128, 96] f32

    def fuse(w):  # [ci, co] -> [2ci, 2co] block-diag over batch
        ci, co = w.shape
        out = np.zeros((2 * ci, 2 * co), np.float32)
        out[:ci, :co] = w
        out[ci:, co:] = w
        return out

    w2bundle = np.concatenate([fuse(W2[0] - W2[2]), fuse(W2[1]), fuse(W2[2])], axis=1)
    return {
        "w1cat": np.ascontiguousarray(w1cat),
        "w2bundle": np.ascontiguousarray(w2bundle.astype(bf)),      # [64, 192]
        "w3a": np.ascontiguousarray(fuse(W3[0] - W3[2]).astype(bf)),  # [64, 256]
        "w3b": np.ascontiguousarray(fuse(W3[1]).astype(bf)),
        "w3c": np.ascontiguousarray(fuse(2.0 * W3[2]).astype(bf)),
        "g1": np.asarray(g1, np.float32)[None, :], "be1": np.asarray(be1, np.float32)[None, :],
        "g2": np.asarray(g2, np.float32)[None, :], "be2": np.asarray(be2, np.float32)[None, :],
        "g3": np.asarray(g3, np.float32)[None, :], "be3": np.asarray(be3, np.float32)[None, :],
    }


def _build_program(meta, debug=False):
    import concourse.bacc as bacc
    import concourse.mybir as mybir
    import concourse.tile as tile
    from concourse.library_config import mlp
    from concourse.masks import make_identity

    f32 = mybir.dt.float32
    bf16 = mybir.dt.bfloat16
    fp8 = mybir.dt.float8e4
    i16 = mybir.dt.int16
    AT = mybir.AluOpType
    stat_dt = fp8 if STAT_FP8 else bf16
    L2g, NCALL, NCH, NCHP, blocks = (
        meta["L2g"], meta["NCALL"], meta["NCH"], meta["NCHP"], meta["blocks"])

    nc = bacc.Bacc("TRN2", target_bir_lowering=False, debug=False, num_devices=NC,
                   num_swdge_queues=NQ,
                   dynamic_dma_scratch_size=int(os.environ.get("BK_SCRATCH", "16384")))

    # ---- I/O ----
    gidx = nc.dram_tensor("gidx", [128, L2g // 16], i16, kind="ExternalInput")
    we_d = nc.dram_tensor("we", [128, NCHP], bf16, kind="ExternalInput")
    wo_d = nc.dram_tensor("wo", [128, NCHP], bf16, kind="ExternalInput")
    dstcol_d = nc.dram_tensor("dstcol", [128, NCHP], f32, kind="ExternalInput")
    iota_d = nc.dram_tensor("iota", [128, 128], f32, kind="ExternalInput")
    xT = nc.dram_tensor("xT", [B, 128, S], f32, kind="ExternalInput")
    xrt = nc.dram_tensor("xrt", [128, SI, 256], bf16, kind="ExternalInput")
    w1cat = nc.dram_tensor("w1cat", [128, 96], f32, kind="ExternalInput")
    w2bundle = nc.dram_tensor("w2bundle", [64, 192], bf16, kind="ExternalInput")
    w3a_d = nc.dram_tensor("w3a", [64, 256], bf16, kind="ExternalInput")
    w3b_d = nc.dram_tensor("w3b", [64, 256], bf16, kind="ExternalInput")
    w3c_d = nc.dram_tensor("w3c", [64, 256], bf16, kind="ExternalInput")
    gbe_w = {"g1": 32, "be1": 32, "g2": 32, "be2": 32, "g3": 128, "be3": 128}
    gbe = {nm: nc.dram_tensor(nm, [1, w], f32, kind="ExternalInput") for nm, w in gbe_w.items()}
    out_d = nc.dram_tensor("out", [128, SI, 256], f32, kind="ExternalOutput")

    # ---- internal DRAM ----
    full = [nc.dram_tensor(f"full{i}", [N, 64], bf16, addr_space="Shared") for i in range(6)]
    shard = [nc.dram_tensor(f"shard{i}", [S, 64], bf16) for i in range(6)]
    st_in = [nc.dram_tensor(f"stin{i}", [1, 512], f32) for i in range(3)]
    st_out = [nc.dram_tensor(f"stout{i}", [1, 512], f32, addr_space="Shared") for i in range(3)]

    RG = [list(range(NC))]

    def shard_tile_ap(i):
        return shard[i][:].rearrange("(p i) e -> p i e", p=128)

    with tile.TileContext(nc) as tc, contextlib.ExitStack() as ctx:
        const = ctx.enter_context(tc.tile_pool(name="const", bufs=1))
        sb = ctx.enter_context(tc.tile_pool(name="sb", bufs=1))
        gp = ctx.enter_context(tc.tile_pool(name="gp", bufs=4))
        rp = ctx.enter_context(tc.tile_pool(name="rp", bufs=4))
        tp2 = ctx.enter_context(tc.tile_pool(name="tp2", bufs=2))
        og = ctx.enter_context(tc.tile_pool(name="og", bufs=2))
        pp = ctx.enter_context(tc.tile_pool(name="pp", bufs=3, space="PSUM"))
        pd = ctx.enter_context(tc.tile_pool(name="pd", bufs=2, space="PSUM"))
        pp1 = ctx.enter_context(tc.tile_pool(name="pp1", bufs=1, space="PSUM"))

        nc.gpsimd.load_library(mlp)

        ident = const.tile([128, 128], bf16)
        make_identity(nc, ident[:])
        ones_k = const.tile([128, 1], f32)
        nc.vector.memset(ones_k[:], 1.0)
        ones_m = const.tile([1, 128], f32)
        nc.vector.memset(ones_m[:], 1.0)
        ones_1 = const.tile([1, 1], f32)
        nc.vector.memset(ones_1[:], 1.0)

        gidx_sb = const.tile([128, L2g // 16], i16)
        we_sb = const.tile([128, NCHP], bf16)
        wo_sb = const.tile([128, NCHP], bf16)
        dcol_sb = const.tile([128, NCHP], f32)
        iota_sb = const.tile([128, 128], f32)
        nc.sync.dma_start(gidx_sb[:], gidx[:])
        nc.sync.dma_start(we_sb[:], we_d[:])
        nc.sync.dma_start(wo_sb[:], wo_d[:])
        nc.sync.dma_start(dcol_sb[:], dstcol_d[:])
        nc.sync.dma_start(iota_sb[:], iota_d[:])

        w1_sb = const.tile([128, 96], f32)
        w2_sb = const.tile([64, 192], bf16)
        w3a = const.tile([64, 256], bf16)
        w3b = const.tile([64, 256], bf16)
        w3c = const.tile([64, 256], bf16)
        nc.sync.dma_start(w1_sb[:], w1cat[:])
        nc.sync.dma_start(w2_sb[:], w2bundle[:])
        nc.sync.dma_start(w3a[:], w3a_d[:])
        nc.sync.dma_start(w3b[:], w3b_d[:])
        nc.sync.dma_start(w3c[:], w3c_d[:])
        gbe_sb = {}
        for nm, w in gbe_w.items():
            t = const.tile([1, w], f32)
            nc.sync.dma_start(t[:], gbe[nm][:])
            gbe_sb[nm] = t

        # ---- one-hot stationaries: built once, SBUF-resident ----
        stat = const.tile([128, NCH, 128], stat_dt)
        for k in range(NCH):
            nc.vector.tensor_scalar(
                out=stat[:, k, :], in0=iota_sb[:], scalar1=dcol_sb[:, k:k + 1],
                scalar2=None, op0=AT.is_equal)

        # ---- propagation ----
        def prop(t_i, prows, post_cb=None):
            """prows: bf16 [128, SI, 64] destination rows (node-major).
            Blocks are processed as soon as their chunks' gathers complete so
            downstream DVE/PE work interleaves with the gather stream.
            post_cb(g, gs): called after each 8-block group's evictions."""
            t2 = full[t_i][:].rearrange("(a b) e -> a (b e)", b=2)  # [N/2, 128] bf16
            Rs = []
            done_b = 0

            def run_block(b):
                k0, k1 = blocks[b]
                ps = pp.tile([128, 64], f32, tag="red")
                for k in range(k0, k1):
                    nc.tensor.matmul(ps[:], lhsT=stat[:, k, :],
                                     rhs=Rs[k // GC][:, k % GC, :],
                                     start=(k == k0), stop=(k == k1 - 1))
                nc.scalar.copy(out=prows[:, b, :], in_=ps[:])
                if post_cb is not None and b % 8 == 7:
                    post_cb(b // 8, slice(b - 7, b + 1))

            for w in range(NCALL):
                G = gp.tile([128, GC, 128], bf16, tag="G")
                nc.gpsimd.dma_gather(G[:], t2,
                                     gidx_sb[:, w * (GCALL // 16):(w + 1) * (GCALL // 16)],
                                     GCALL, GCALL, 128, queue_num=w % NQ)
                ws = slice(w * GC, (w + 1) * GC)
                T = tp2.tile([128, GC, 64], bf16, tag="T")
                nc.vector.tensor_tensor(
                    out=T[:], in0=G[:, :, 64:128],
                    in1=wo_sb[:, ws, None].to_broadcast([128, GC, 64]), op=AT.mult)
                R = rp.tile([128, GC, 64], bf16, tag="R")
                nc.vector.tensor_tensor(
                    out=R[:], in0=G[:, :, 0:64],
                    in1=we_sb[:, ws, None].to_broadcast([128, GC, 64]), op=AT.mult)
                nc.vector.tensor_tensor(out=R[:], in0=R[:], in1=T[:], op=AT.add)
                Rs.append(R)
                hi = (w + 1) * GC
                while done_b < SI and blocks[done_b][1] <= hi:
                    run_block(done_b)
                    done_b += 1
            while done_b < SI:
                run_block(done_b)
                done_b += 1

        # ---- BatchNorm helpers ----
        def bn_sf(sums, cmid, g_t, be_t, st_i):
            F = 2 * cmid
            ps = pp1.tile([1, 512], f32, tag="bnps")
            nc.tensor.matmul(ps[:, 0:2 * F], lhsT=ones_k[:], rhs=sums[:, 0:2 * F],
                             start=True, stop=True)
            stt = sb.tile([1, 512], f32, tag="bnstt")
            nc.vector.tensor_copy(out=stt[:, 0:2 * F], in_=ps[:, 0:2 * F])
            if 2 * F < 512:
                nc.vector.memset(stt[:, 2 * F:], 0.0)
            nc.sync.dma_start(st_in[st_i][:], stt[:])
            nc.gpsimd.collective_compute(
                "AllReduce", AT.add, replica_groups=RG,
                ins=[st_in[st_i][:].opt()], outs=[st_out[st_i][:].opt()])
            stf = sb.tile([1, 512], f32, tag="bnstf")
            nc.sync.dma_start(stf[:], st_out[st_i][:])
            cs = sb.tile([1, 8 * cmid], f32, tag="bncs")
            nc.vector.tensor_tensor(out=cs[:, 0:cmid], in0=stf[:, 0:cmid],
                                    in1=stf[:, cmid:F], op=AT.add)
            nc.vector.tensor_tensor(out=cs[:, cmid:2 * cmid], in0=stf[:, F:F + cmid],
                                    in1=stf[:, F + cmid:2 * F], op=AT.add)
            inv_n = 1.0 / float(B * N)
            mu = cs[:, 4 * cmid:5 * cmid]
            nc.vector.tensor_scalar_mul(mu, cs[:, 0:cmid], inv_n)
            msq = cs[:, 5 * cmid:6 * cmid]
            nc.vector.tensor_scalar_mul(msq, cs[:, cmid:2 * cmid], inv_n)
            var = cs[:, 6 * cmid:7 * cmid]
            nc.vector.tensor_tensor(out=var, in0=mu, in1=mu, op=AT.mult)
            nc.vector.tensor_tensor(out=var, in0=msq, in1=var, op=AT.subtract)
            nc.vector.tensor_scalar_add(var, var, EPS)
            std = cs[:, 7 * cmid:8 * cmid]
            nc.scalar.sqrt(std, var)
            rstd = cs[:, 6 * cmid:7 * cmid]
            nc.vector.reciprocal(rstd, std)
            s_ = cs[:, 2 * cmid:3 * cmid]
            nc.vector.tensor_tensor(out=s_, in0=g_t[:], in1=rstd, op=AT.mult)
            o_ = cs[:, 3 * cmid:4 * cmid]
            nc.vector.tensor_tensor(out=o_, in0=mu, in1=s_, op=AT.mult)
            nc.vector.tensor_tensor(out=o_, in0=be_t[:], in1=o_, op=AT.subtract)
            sf = sb.tile([1, 512], f32, tag="bnsf")
            nc.vector.tensor_copy(out=sf[:, 0:cmid], in_=s_)
            nc.vector.tensor_copy(out=sf[:, cmid:F], in_=s_)
            nc.vector.tensor_copy(out=sf[:, F:F + cmid], in_=o_)
            nc.vector.tensor_copy(out=sf[:, F + cmid:2 * F], in_=o_)
            return sf

        def bn_rep(sf, F):
            psb = pp1.tile([128, 512], f32, tag="bnps")
            nc.tensor.matmul(psb[:, 0:2 * F], lhsT=ones_m[:], rhs=sf[:, 0:2 * F],
                             start=True, stop=True)
            rep = sb.tile([128, 512], f32, tag="bnrep")
            nc.vector.tensor_copy(out=rep[:, 0:2 * F], in_=psb[:, 0:2 * F])
            return rep

        def bn_cols(sf, F):
            # per-partition BN coeff columns for channel-major apply
            psc = pp1.tile([F, 2], f32, tag="bnps")
            nc.tensor.matmul(psc[:, 0:1], lhsT=sf[:, 0:F], rhs=ones_1[:],
                             start=True, stop=True)
            nc.tensor.matmul(psc[:, 1:2], lhsT=sf[:, F:2 * F], rhs=ones_1[:],
                             start=True, stop=True)
            cols = sb.tile([F, 2], f32, tag="bncols")
            nc.scalar.copy(out=cols[:], in_=psc[:])
            return cols

        def make_stats_cb(prows, a_view, sums, fused4=False, t_dst=None):
            """o = p + a in place on prows; accumulate stats; optionally
            transpose the o tiles into t_dst ([64|128]-partition slices)."""
            def cb(g, gs):
                if fused4:
                    nc.vector.tensor_tensor(
                        out=prows[:, gs, :].rearrange("p i (b c) -> p i b c", b=2),
                        in0=prows[:, gs, :].rearrange("p i (b c) -> p i b c", b=2),
                        in1=a_view[:, gs], op=AT.add)
                else:
                    nc.vector.tensor_tensor(out=prows[:, gs, :], in0=prows[:, gs, :],
                                            in1=a_view[:, gs], op=AT.add)
                red1 = tp2.tile([128, 64], f32, tag="bnred2")
                nc.vector.tensor_reduce(
                    out=red1[:], in_=prows[:, gs, :].rearrange("p i c -> p c i"),
                    axis=mybir.AxisListType.X, op=AT.add)
                nc.vector.tensor_tensor(out=sums[:, 0:64], in0=sums[:, 0:64],
                                        in1=red1[:], op=AT.add)
                sq = tp2.tile([128, 8, 64], bf16, tag="bnsq")
                nc.vector.tensor_tensor(out=sq[:], in0=prows[:, gs, :],
                                        in1=prows[:, gs, :], op=AT.mult)
                red2 = tp2.tile([128, 64], f32, tag="bnred2")
                nc.vector.tensor_reduce(
                    out=red2[:], in_=sq[:].rearrange("p i c -> p c i"),
                    axis=mybir.AxisListType.X, op=AT.add)
                nc.vector.tensor_tensor(out=sums[:, 64:128], in0=sums[:, 64:128],
                                        in1=red2[:], op=AT.add)
                if t_dst is not None:
                    for i in range(gs.start, gs.stop):
                        tp = pd.tile([64, 128], bf16, tag="tps")
                        nc.tensor.transpose(out=tp[:], in_=prows[:, i, :],
                                            identity=ident[:])
                        nc.scalar.copy(out=t_dst[:, i, :], in_=tp[:])
            return cb

        def transpose_cb(srcs_dsts):
            """cb transposing [128,64] tiles of (src, dst-slice) pairs."""
            def cb(g, gs):
                for src_t, dst_t in srcs_dsts:
                    for i in range(gs.start, gs.stop):
                        tp = pd.tile([64, 128], bf16, tag="tps")
                        nc.tensor.transpose(out=tp[:], in_=src_t[:, i, :],
                                            identity=ident[:])
                        nc.scalar.copy(out=dst_t[:, i, :], in_=tp[:])
            return cb

        def bn_apply(orows, sums, cmid, g_t, be_t, st_i):
            F = 2 * cmid
            rep = bn_rep(bn_sf(sums, cmid, g_t, be_t, st_i), F)
            zr = sb.tile([128, SI, F], bf16, tag="Z")
            nc.vector.tensor_tensor(out=zr[:], in0=orows[:],
                                    in1=rep[:, None, 0:F].to_broadcast([128, SI, F]), op=AT.mult)
            nc.vector.tensor_tensor(out=zr[:], in0=zr[:],
                                    in1=rep[:, None, F:2 * F].to_broadcast([128, SI, F]), op=AT.add)
            nc.vector.tensor_scalar_max(zr[:], zr[:], 0.0)
            return zr

        # ================= Layer 1 dense (f32 in, bf16 out) =================
        # v-pass first so AllGather0 starts early; a/u pass runs under it.
        vt = sb.tile([128, SI, 2, 32], bf16, tag="V")
        for g in range(SI // 8):
            for b in range(B):
                xtb = og.tile([128, 1024], f32, tag="xtb")
                nc.sync.dma_start(xtb[:], xT[b, :, g * 1024:(g + 1) * 1024])
                for j in range(8):
                    i = g * 8 + j
                    psd = pd.tile([128, 256], f32, tag="dps")
                    nc.tensor.matmul(psd[:, 0:32], lhsT=xtb[:, j * 128:(j + 1) * 128],
                                     rhs=w1_sb[:, 64:96], start=True, stop=True)
                    nc.scalar.copy(out=vt[:, i, b, :], in_=psd[:, 0:32])
        for b in range(B):
            nc.sync.dma_start(shard_tile_ap(0)[:, :, b * 32:(b + 1) * 32],
                              vt[:, :, b, :])
        nc.gpsimd.collective_compute("AllGather", AT.bypass, replica_groups=RG,
                                     ins=[shard[0][:].opt()], outs=[full[0][:].opt()])
        # a/u pass: hold1 [128, SI, 2, 64] = [a32 | u32] per (tile, batch)
        hold1 = sb.tile([128, SI, 2, 64], bf16, tag="hold")
        for g in range(SI // 8):
            for b in range(B):
                xtb = og.tile([128, 1024], f32, tag="xtb")
                nc.sync.dma_start(xtb[:], xT[b, :, g * 1024:(g + 1) * 1024])
                for j in range(8):
                    i = g * 8 + j
                    psd = pd.tile([128, 256], f32, tag="dps")
                    nc.tensor.matmul(psd[:, 0:64], lhsT=xtb[:, j * 128:(j + 1) * 128],
                                     rhs=w1_sb[:, 0:64], start=True, stop=True)
                    nc.scalar.copy(out=hold1[:, i, b, :], in_=psd[:, 0:64])
        p11 = sb.tile([128, SI, 64], bf16, tag="P")
        prop(0, p11)
        # q1 in place on p11
        nc.vector.scalar_tensor_tensor(
            out=p11[:].rearrange("p i (b c) -> p i b c", b=2),
            in0=p11[:].rearrange("p i (b c) -> p i b c", b=2),
            scalar=2.0, in1=hold1[:, :, :, 32:64], op0=AT.mult, op1=AT.add)
        nc.sync.dma_start(shard_tile_ap(1), p11[:])
        nc.gpsimd.collective_compute("AllGather", AT.bypass, replica_groups=RG,
                                     ins=[shard[1][:].opt()], outs=[full[1][:].opt()])
        sums1 = sb.tile([128, 512], f32, tag="bnsums")
        nc.vector.memset(sums1[:, 0:128], 0.0)
        # channel-major transposed tiles (partitions 0:64):
        # TTa: o1T/z2T then z3T; TTb: t1rT.
        TTa = sb.tile([64, SI, 128], bf16, tag="TTa")
        TTb = sb.tile([64, SI, 128], bf16, tag="TTb")
        p12 = sb.tile([128, SI, 64], bf16, tag="P")
        prop(1, p12, post_cb=make_stats_cb(p12, hold1[:, :, :, 0:32], sums1,
                                           fused4=True, t_dst=TTa))
        sf1 = bn_sf(sums1, C_MID, gbe_sb["g1"], gbe_sb["be1"], 0)
        cols1 = bn_cols(sf1, 2 * C_MID)
        # z2T = relu(s*o1T + t) in place, channel-major, one batched op
        nc.scalar.activation(out=TTa[:], in_=TTa[:],
                             func=mybir.ActivationFunctionType.Relu,
                             bias=cols1[:, 1:2], scale=cols1[:, 0:1])

        # ================= Layer 2 (bf16, channel-major dense) =================
        # v-pass
        vt2 = sb.tile([128, SI, 64], bf16, tag="V")
        for i in range(SI):
            psd = pd.tile([128, 256], f32, tag="dps")
            nc.tensor.matmul(psd[:, 0:64], lhsT=TTa[:, i, :], rhs=w2_sb[:, 128:192],
                             start=True, stop=True)
            nc.scalar.copy(out=vt2[:, i, :], in_=psd[:, 0:64])
        nc.sync.dma_start(shard_tile_ap(2), vt2[:])
        nc.gpsimd.collective_compute("AllGather", AT.bypass, replica_groups=RG,
                                     ins=[shard[2][:].opt()], outs=[full[2][:].opt()])
        # a/u pass
        hold2 = sb.tile([128, SI, 128], bf16, tag="hold")
        for i in range(SI):
            psd = pd.tile([128, 256], f32, tag="dps")
            nc.tensor.matmul(psd[:, 0:128], lhsT=TTa[:, i, :], rhs=w2_sb[:, 0:128],
                             start=True, stop=True)
            nc.scalar.copy(out=hold2[:, i, :], in_=psd[:, 0:128])
        p21 = sb.tile([128, SI, 64], bf16, tag="P")
        prop(2, p21)
        nc.vector.scalar_tensor_tensor(out=p21[:], in0=p21[:], scalar=2.0,
                                       in1=hold2[:, :, 64:128], op0=AT.mult, op1=AT.add)
        nc.sync.dma_start(shard_tile_ap(3), p21[:])
        nc.gpsimd.collective_compute("AllGather", AT.bypass, replica_groups=RG,
                                     ins=[shard[3][:].opt()], outs=[full[3][:].opt()])
        sums2 = sb.tile([128, 512], f32, tag="bnsums")
        nc.vector.memset(sums2[:, 0:128], 0.0)
        p22 = sb.tile([128, SI, 64], bf16, tag="P")
        prop(3, p22, post_cb=make_stats_cb(p22, hold2[:, :, 0:64], sums2))
        z3 = bn_apply(p22, sums2, C_MID, gbe_sb["g2"], gbe_sb["be2"], 1)

        # ================= Layer 3 =================
        nc.sync.dma_start(shard_tile_ap(4), z3[:])
        nc.gpsimd.collective_compute("AllGather", AT.bypass, replica_groups=RG,
                                     ins=[shard[4][:].opt()], outs=[full[4][:].opt()])
        t1r = sb.tile([128, SI, 64], bf16, tag="P")
        prop(4, t1r, post_cb=transpose_cb([(z3, TTa), (t1r, TTb)]))
        nc.sync.dma_start(shard_tile_ap(5), t1r[:])
        nc.gpsimd.collective_compute("AllGather", AT.bypass, replica_groups=RG,
                                     ins=[shard[5][:].opt()], outs=[full[5][:].opt()])
        p32 = sb.tile([128, SI, 64], bf16, tag="Q")
        o3 = sb.tile([128, SI, 256], bf16, tag="O3")
        acc_s = sb.tile([128, 512], f32, tag="bnsums")
        nc.vector.memset(acc_s[:], 0.0)

        def l3_cb(g, gs):
            for i in range(gs.start, gs.stop):
                psd = pd.tile([128, 256], f32, tag="dps")
                nc.tensor.matmul(psd[:], lhsT=TTa[:, i, :], rhs=w3a[:],
                                 start=True, stop=False)
                nc.tensor.matmul(psd[:], lhsT=TTb[:, i, :], rhs=w3b[:],
                                 start=False, stop=False)
                tp = pd.tile([64, 128], bf16, tag="tps")
                nc.tensor.transpose(out=tp[:], in_=p32[:, i, :], identity=ident[:])
                ztc = tp2.tile([64, 128], bf16, tag="ztc")
                nc.scalar.copy(out=ztc[:], in_=tp[:])
                nc.tensor.matmul(psd[:], lhsT=ztc[:], rhs=w3c[:], start=False, stop=True)
                nc.scalar.copy(out=o3[:, i, :], in_=psd[:])
            red = sb.tile([128, 512], f32, tag="bnred")
            nc.vector.tensor_reduce(out=red[:, 0:256],
                                    in_=o3[:, gs, :].rearrange("p j c -> p c j"),
                                    axis=mybir.AxisListType.X, op=AT.add)
            nc.vector.memset(red[:, 256:512], 0.0)
            for h in range(2):
                sqh = og.tile([128, 4, 256], bf16, tag="o3sq")
                nc.vector.tensor_tensor(out=sqh[:], in0=o3[:, gs.start + h * 4:gs.start + (h + 1) * 4, :],
                                        in1=o3[:, gs.start + h * 4:gs.start + (h + 1) * 4, :], op=AT.mult)
                red2 = sb.tile([128, 256], f32, tag="bnred2b")
                nc.vector.tensor_reduce(out=red2[:], in_=sqh[:].rearrange("p j c -> p c j"),
                                        axis=mybir.AxisListType.X, op=AT.add)
                nc.vector.tensor_tensor(out=red[:, 256:512], in0=red[:, 256:512],
                                        in1=red2[:], op=AT.add)
            nc.vector.tensor_tensor(out=acc_s[:], in0=acc_s[:], in1=red[:], op=AT.add)

        prop(5, p32, post_cb=l3_cb)
        rep3 = bn_rep(bn_sf(acc_s, C_OUT, gbe_sb["g3"], gbe_sb["be3"], 2), 2 * C_OUT)

        for g in range(SI // 2):
            gs = slice(g * 2, (g + 1) * 2)
            zc = og.tile([128, 2, 256], f32, tag="zc")
            nc.vector.tensor_tensor(out=zc[:], in0=o3[:, gs, :],
                                    in1=rep3[:, None, 0:256].to_broadcast([128, 2, 256]),
                                    op=AT.mult)
            nc.vector.tensor_tensor(out=zc[:], in0=zc[:],
                                    in1=rep3[:, None, 256:512].to_broadcast([128, 2, 256]),
                                    op=AT.add)
            nc.scalar.activation(out=zc[:], in_=zc[:],
                                 func=mybir.ActivationFunctionType.Relu)
            xc = og.tile([128, 2, 256], bf16, tag="xc")
            nc.sync.dma_start(xc[:], xrt[:, gs, :])
            nc.vector.tensor_tensor(out=zc[:], in0=zc[:], in1=xc[:], op=AT.add)
            nc.scalar.activation(out=zc[:], in_=zc[:],
                                 func=mybir.ActivationFunctionType.Relu)
            nc.sync.dma_start(out_d[:, gs, :], zc[:])

    nc.compile()
    return nc


def kernel(x, edge_index, edge_weight,
           W1, b1, g1, be1, W2, b2, g2, be2, W3, b3, g3, be3):
    from concourse.bass_utils import run_bass_kernel_spmd

    x = np.asarray(x, np.float32)
    in_maps, meta = _host_prep(x, edge_index, edge_weight)
    wts = _pack_weights(W1, W2, W3, g1, be1, g2, be2, g3, be3)
    for m in in_maps:
        m.update(wts)

    key = (meta["L2g"], meta["NCH"], tuple(k for _, k in meta["blocks"]))
    if key not in _CACHE:
        _CACHE[key] = _build_program(meta)
    nc = _CACHE[key]

    trace = os.environ.get("BK_TRACE", "0") == "1"
    kw = {"trace": True} if trace else {}
    res = run_bass_kernel_spmd(nc, in_maps, list(range(NC)), **kw)
    if trace:
        print(f"HW exec time: {res.exec_time_ns} ns (mean {res.mean_exec_time_ns})")

    out = np.empty((B, N, 128), np.float32)
    assign = meta["assign"]
    for c in range(NC):
        oc = res.results[c]["out"]  # [128, SI, 256] tile layout
        rows = oc.transpose(1, 0, 2).reshape(SI, 128, 256)  # [pos, p, 256]
        node_ids = (assign[c][:, None] * 128 + np.arange(128)[None, :])
        out[0, node_ids.reshape(-1), :] = rows.reshape(S, 256)[:, 0:128]
        out[1, node_ids.reshape(-1), :] = rows.reshape(S, 256)[:, 128:256]
    kernel._last_results = res
    return out
